# revision 1
# baseline (speedup 1.0000x reference)
"""Trainium2 Bass kernel for nn_AZConv2d (fuzzy-rule hyperbolic-geometry message passing).

Self-contained: hardcodes shapes B=8,C=64,H=W=128,R=4,Cout=64; shards batch over 8 cores.

v2: stencil restructured for engine balance.
  - Phase A: per image row, two PSUM-accumulating bf16 matmuls with the stacked
    slab [x_hi(64); x_lo(64)] as weights: M1 (272 cols) gives w_hi*x for all
    features (+ w_hi*x_lo for the 16 gate/geom cols), M2 (16 cols) adds
    w_lo*x_hi for gate/geom -> gq is f32-accurate (theta degeneracy needs it),
    z stays bf16. Biases are NOT in the matmul (no ones row): they enter as
    per-rule [P,1] bias APs in the field ops; pw_b via a rank-1 matmul.
  - gq: [128, 16, 130] f32; z: zbuf [128, 256, 130] bf16 with zero halo cols.
  - Fields/kern/compat/den/w: as v1 (f32 DVE/ACT small ops).
  - Stencil: per 8-row block, 9 large DVE tensor_tensor multiplies (bf16 2x mode,
    weight broadcast over o via stride-0 AP): P_dx[g,dy,r,o,rho] = w'[g,dy,r,rho]*z.
    Weights pre-shifted across partitions by -dx (tiny PE matmuls) so products
    live on the z partition; the 36-term (dx,dy,r) sum runs on the Tensor engine
    as PSUM-accumulating matmuls with shift/identity matrices (C[g] = P[g+dx]).
    Row (dy) offsets fold into the product APs via the z halo columns.
  - Out-of-range taps have w == 0 exactly (mu zero-padding), so halo/edge
    garbage is annihilated; PSUM evacuated by ACT, DMA'd per block.
"""
import numpy as np
from contextlib import ExitStack

import concourse.bass as bass
import concourse.tile as tile
from concourse import mybir
from concourse.bass_utils import run_bass_kernel_spmd

F32 = mybir.dt.float32
BF16 = mybir.dt.bfloat16
AF = mybir.ActivationFunctionType
OP = mybir.AluOpType

B, C, H, W, R, Cout = 8, 64, 128, 128, 4, 64
L = H * W
NCORE = 8
NFEAT = 16 + R * Cout  # 272
STRIP = 32
NSTRIP = H // STRIP
SW = STRIP + 2          # field window rows per strip
BLK = 8                 # stencil rows per psum accumulation block
PAIRS = [(0, 1), (1, -1), (1, 0), (1, 1)]
HALF_PI = float(np.pi / 2)

_CACHE = {}


def split_multiwaits(nc):
    """This walrus accepts ONE sync wait per instruction: split extras into
    same-engine NoOps inserted just before the instruction."""
    n = 0
    for bb in nc.main_func.blocks:
        out = []
        for ins in bb.instructions:
            si = ins.sync_info
            if si is not None and len(si.on_wait) > 1:
                waits = list(si.on_wait)
                for w in waits[:-1]:
                    n += 1
                    nop = mybir.InstNoOp(name=f"WSPLIT-{n}")
                    nop.engine = ins.engine
                    nop.sync_info = mybir.SyncInfo(on_wait=[w], on_update=[])
                    out.append(nop)
                ins.sync_info = mybir.SyncInfo(on_wait=[waits[-1]],
                                               on_update=list(si.on_update))
            out.append(ins)
        bb.instructions[:] = out
    return n


def build_program(for_sim=False, debug=False):
    nc = bass.Bass()
    xf_d = nc.dram_tensor("xf", [C, L], F32, kind="ExternalInput")
    xh_d = nc.dram_tensor("xh", [C, L], BF16, kind="ExternalInput")
    w1_d = nc.dram_tensor("w1", [C, 256], BF16, kind="ExternalInput")
    w2_d = nc.dram_tensor("w2", [C, 16], F32, kind="ExternalInput")
    smat_d = nc.dram_tensor("smat", [128, 256], F32, kind="ExternalInput")
    smatbf_d = nc.dram_tensor("smatbf", [128, 384], BF16, kind="ExternalInput")
    aux_d = nc.dram_tensor("aux", [1, 640], BF16, kind="ExternalInput")
    cbias_d = nc.dram_tensor("cbias", [128, 40], F32, kind="ExternalInput")
    out_d = nc.dram_tensor("out", [L, Cout], F32, kind="ExternalOutput")
    dbg = None
    if debug:
        dbg = {
            "dbg_gq": nc.dram_tensor("dbg_gq", [128, 16, H + 2], F32,
                                     kind="ExternalOutput")[:],
            "dbg_z": nc.dram_tensor("dbg_z", [128, 256, H + 2], BF16,
                                    kind="ExternalOutput")[:],
            "dbg_wt": nc.dram_tensor("dbg_wt", [128, NSTRIP, 3, 3, R, STRIP],
                                     BF16, kind="ExternalOutput")[:],
            "dbg_mu": nc.dram_tensor("dbg_mu", [128, R, H + 2], F32,
                                     kind="ExternalOutput")[:],
            "dbg_den": nc.dram_tensor("dbg_den", [128, NSTRIP, R, STRIP], F32,
                                      kind="ExternalOutput")[:],
        }

    with ExitStack() as ctx:
        tc = ctx.enter_context(tile.TileContext(nc))
        _emit(ctx, tc, xf_d[:], xh_d[:], w1_d[:], w2_d[:], smat_d[:],
              smatbf_d[:], aux_d[:], cbias_d[:], out_d[:], dbg)
    if not for_sim:
        split_multiwaits(nc)
    return nc


def _emit(ctx, tc, xf_d, xh_d, w1_d, w2_d, smat_d, smatbf_d, aux_d, cbias_d,
          out_d, dbg=None):
    nc = tc.nc

    persist = ctx.enter_context(tc.tile_pool(name="persist", bufs=1))
    psumA = ctx.enter_context(tc.tile_pool(name="psumA", bufs=2, space="PSUM"))
    psumB = ctx.enter_context(tc.tile_pool(name="psumB", bufs=2, space="PSUM"))
    psum_sh = ctx.enter_context(tc.tile_pool(name="psum_sh", bufs=1, space="PSUM"))
    strip_pool = ctx.enter_context(tc.tile_pool(name="strip", bufs=2))
    pairtmp = ctx.enter_context(tc.tile_pool(name="pairtmp", bufs=1))

    # ---------------- persistent tensors ----------------
    w1_sb = persist.tile([C, 256], BF16)
    nc.sync.dma_start(out=w1_sb, in_=w1_d)
    w2_sb = persist.tile([C, 16], F32)
    nc.sync.dma_start(out=w2_sb, in_=w2_d)
    smat = persist.tile([128, 256], F32)       # [Sp | Sm]
    nc.sync.dma_start(out=smat, in_=smat_d)
    smat_bf = persist.tile([128, 384], BF16)   # [Sp | Sm | I]
    nc.sync.dma_start(out=smat_bf, in_=smatbf_d)
    SHIFT = {1: smat[:, 0:128], -1: smat[:, 128:256]}
    SHIFT_BF = {1: smat_bf[:, 0:128], -1: smat_bf[:, 128:256],
                0: smat_bf[:, 256:384]}
    aux_bf = persist.tile([1, 640], BF16)      # [ones(128) | pwb_row(512)]
    nc.sync.dma_start(out=aux_bf, in_=aux_d)
    cbias = persist.tile([128, 40], F32)
    nc.sync.dma_start(out=cbias, in_=cbias_d)

    def cbs(col):
        return cbias[:, col:col + 1]

    # bias constants for ACT ops
    cb = persist.tile([128, 4], F32)
    nc.vector.memset(cb[:, 0:1], 1e-30)
    nc.vector.memset(cb[:, 1:2], 2e-4)
    nc.vector.memset(cb[:, 2:3], 1e-6)
    nc.vector.memset(cb[:, 3:4], HALF_PI)

    # gq: [128, 16 fields, 130] f32; zbuf: [128, 256, 130] bf16, zero halo.
    gq = persist.tile([128, 16, H + 2], F32)
    nc.vector.memset(gq[:, :, 0], 0.0)
    nc.vector.memset(gq[:, :, H + 1], 0.0)
    zbuf = persist.tile([128, 256, H + 2], BF16)
    nc.vector.memset(zbuf[:, :, 0], 0.0)
    nc.vector.memset(zbuf[:, :, H + 1], 0.0)
    zview = zbuf.rearrange("p (r o) c -> p r o c", r=R)

    # full-image small planes [128, R, H+2]
    mu = persist.tile([128, R, H + 2], F32)
    mup = persist.tile([128, R, H + 2], BF16)  # mu[g+1] (zero pad)
    mum = persist.tile([128, R, H + 2], BF16)  # mu[g-1]
    c2cF = persist.tile([128, R, H + 2], F32)
    s2cF = persist.tile([128, R, H + 2], F32)
    comu = [persist.tile([128, R, H + 2], BF16, name=f"comu{i}")
            for i in range(len(PAIRS))]

    # ---------------- phase A: projections (x streamed, 2 matmuls per row:
    # z from bf16 x, gq from true f32 x for theta-degeneracy accuracy) ----
    with tc.tile_pool(name="phA", bufs=2) as pha:
        for k in range(NSTRIP):
            q0 = k * STRIP
            xwh = pha.tile([C, STRIP * 128], BF16, tag="xwinh")
            nc.sync.dma_start(out=xwh, in_=xh_d[:, q0 * 128:(q0 + STRIP) * 128])
            xwf = pha.tile([C, STRIP * 128], F32, tag="xwinf")
            nc.sync.dma_start(out=xwf, in_=xf_d[:, q0 * 128:(q0 + STRIP) * 128])
            for j in range(STRIP):
                rho = q0 + j
                pt = psumA.tile([128, NFEAT], F32, tag="proj")
                ptg, ptz = pt[:, 0:16], pt[:, 16:NFEAT]
                nc.tensor.matmul(ptz, xwh[:, j * 128:(j + 1) * 128], w1_sb,
                                 start=True, stop=True, skip_group_check=True)
                nc.tensor.matmul(ptg, xwf[:, j * 128:(j + 1) * 128], w2_sb,
                                 start=True, stop=True, skip_group_check=True)
                if j % 2 == 0:
                    nc.scalar.activation(zbuf[:, :, 1 + rho], ptz, AF.Copy)
                    nc.vector.tensor_copy(gq[:, :, 1 + rho], ptg)
                else:
                    nc.vector.tensor_copy(zbuf[:, :, 1 + rho], ptz)
                    nc.scalar.activation(gq[:, :, 1 + rho], ptg, AF.Copy)

    if dbg is not None:
        nc.sync.dma_start(out=dbg["dbg_gq"], in_=gq)
        nc.sync.dma_start(out=dbg["dbg_z"], in_=zbuf)

    # ---------------- phase B: full-image fields ----------------
    # cbias cols: 0:4 gate_b | 4:8 b_th | 8:12 2*b_th | 12:16 2*b_th+pi/2
    #   16:20 -pi/2-b_th | 20:24 pi/2-b_th | 24:28 -3pi/4-b_th | 28:32 pi/4-b_th
    #   32:36 b_base | 36:40 b_hyp
    PI = float(np.pi)
    with tc.tile_pool(name="phB", bufs=1) as phb:
        eg = phb.tile([128, R, H + 2], F32, tag="eg")
        for r in range(R):
            nc.scalar.activation(eg[:, r, 1:H + 1], gq[:, r, 1:H + 1], AF.Exp,
                                 bias=cbs(0 + r))
        for gcol in (0, H + 1):
            nc.vector.memset(eg[:, :, gcol], 0.0)
        zsum = phb.tile([128, H + 2], F32, tag="zsum")
        nc.vector.tensor_tensor(zsum, eg[:, 0, :], eg[:, 1, :], op=OP.add)
        nc.vector.tensor_tensor(zsum, zsum, eg[:, 2, :], op=OP.add)
        nc.vector.tensor_tensor(zsum, zsum, eg[:, 3, :], op=OP.add)
        rz = phb.tile([128, H + 2], F32, tag="rz")
        nc.scalar.activation(rz, zsum, AF.Ln, bias=cb[:, 0:1])
        nc.scalar.activation(rz, rz, AF.Exp, scale=-1.0)
        for r in range(R):
            nc.vector.tensor_tensor(mu[:, r, :], eg[:, r, :], rz, op=OP.mult)

        # Sin table is only valid on [-pi, pi]; range-reduce 2*(theta+b) with
        # one +-2pi correction (theta+b range here is within +-3.7).
        m1 = phb.tile([128, R, H + 2], F32, tag="m1")
        m2 = phb.tile([128, R, H + 2], F32, tag="m2")
        tred = phb.tile([128, R, H + 2], F32, tag="tred")
        for r in range(R):
            thr = gq[:, 4 + r, :]
            # s2cF_r = sin(2*(th+b) + 2pi*d), d = [th < -pi/2-b] - [th > pi/2-b]
            nc.vector.tensor_scalar(m1[:, r], thr, cbs(16 + r), None, op0=OP.is_lt)
            nc.vector.tensor_scalar(m2[:, r], thr, cbs(20 + r), None, op0=OP.is_gt)
            nc.vector.tensor_tensor(m1[:, r], m1[:, r], m2[:, r], op=OP.subtract)
            nc.vector.scalar_tensor_tensor(out=tred[:, r], in0=m1[:, r],
                                           scalar=PI, in1=thr,
                                           op0=OP.mult, op1=OP.add)
            nc.scalar.activation(s2cF[:, r], tred[:, r], AF.Sin, scale=2.0,
                                 bias=cbs(8 + r))
            # c2cF_r = sin(2*(th+b) + pi/2 + 2pi*dc)
            nc.vector.tensor_scalar(m1[:, r], thr, cbs(24 + r), None, op0=OP.is_lt)
            nc.vector.tensor_scalar(m2[:, r], thr, cbs(28 + r), None, op0=OP.is_gt)
            nc.vector.tensor_tensor(m1[:, r], m1[:, r], m2[:, r], op=OP.subtract)
            nc.vector.scalar_tensor_tensor(out=tred[:, r], in0=m1[:, r],
                                           scalar=PI, in1=thr,
                                           op0=OP.mult, op1=OP.add)
            nc.scalar.activation(c2cF[:, r], tred[:, r], AF.Sin, scale=2.0,
                                 bias=cbs(12 + r))

    if dbg is not None:
        nc.sync.dma_start(out=dbg["dbg_mu"], in_=mu)

    for ip in range(len(PAIRS)):
        nc.vector.memset(comu[ip], 0.0)

    # mu shifted copies via PE (N=520 > 512 -> two chunks of 260)
    for sgn, dst in ((1, mup), (-1, mum)):
        for h in range(2):
            mq = psum_sh.tile([128, 2, H + 2], F32, tag="mush")
            nc.tensor.matmul(mq, SHIFT[sgn], mu[:, 2 * h:2 * h + 2, :],
                             start=True, stop=True)
            nc.scalar.activation(dst[:, 2 * h:2 * h + 2, :], mq, AF.Copy)

    ppool = ctx.enter_context(tc.tile_pool(name="ppool", bufs=2))

    # ---------------- phase C per strip ----------------
    for k in range(NSTRIP):
        q0 = k * STRIP

        # strip field tiles [128, R, SW]; window col j = image row q0-1+j
        c2c = c2cF[:, :, q0:q0 + SW]
        s2c = s2cF[:, :, q0:q0 + SW]
        uh = strip_pool.tile([128, R, SW], F32, tag="uh")    # e^{raw_hyper+b}
        Ft = strip_pool.tile([128, R, SW], F32, tag="Ft")    # 1+uh, DVE only
        Gt = strip_pool.tile([128, R, SW], F32, tag="Gt")    # e^{-softplus}
        bt = strip_pool.tile([128, R, SW], F32, tag="bt")    # softplus(raw_base+b)
        ub = strip_pool.tile([128, R, SW], F32, tag="ub")
        for r in range(R):
            nc.scalar.activation(uh[:, r], gq[:, 12 + r, q0:q0 + SW], AF.Exp,
                                 bias=cbs(36 + r))
            nc.scalar.activation(ub[:, r], gq[:, 8 + r, q0:q0 + SW], AF.Exp,
                                 bias=cbs(32 + r))
        nc.vector.tensor_scalar_add(Ft, uh, 1.0)
        nc.scalar.activation(Gt, uh, AF.Ln, bias=1.0)
        nc.scalar.activation(Gt, Gt, AF.Exp, scale=-1.0)
        nc.scalar.activation(bt, ub, AF.Ln, bias=1.0)

        # dx-shifted field copies via PE shift matmuls (zero-padded at edges;
        # pad values only feed taps where mu_n = 0, any finite value is fine)
        shifted = {}
        for name, t in (("c2c", c2c), ("s2c", s2c), ("uh", uh), ("Gt", Gt),
                        ("bt", bt)):
            d = {0: t}
            for sgn in (1, -1):
                ps = psum_sh.tile([128, R, SW], F32, tag="fsh")
                nc.tensor.matmul(ps, SHIFT[sgn], t, start=True, stop=True)
                st = strip_pool.tile([128, R, SW], F32, tag=f"{name}s{sgn}")
                if name == "uh":
                    # evac with +1 fused: shifted F = shifted(uh) + 1
                    nc.vector.tensor_scalar_add(st, ps, 1.0)
                else:
                    nc.scalar.activation(st, ps, AF.Copy)
                d[sgn] = st
            shifted[name] = d
        shifted["Ft"] = {0: Ft, 1: shifted["uh"][1], -1: shifted["uh"][-1]}

        # denominator accumulator; init with center compat (= mu)
        den = strip_pool.tile([128, R, STRIP], F32, tag="den")
        nc.vector.tensor_copy(den, mu[:, :, 1 + q0:1 + q0 + STRIP])

        compat_t = {}
        comu_sh_t = {}
        for ip, (dy, dx) in enumerate(PAIRS):
            def S(name):
                return shifted[name][dx][:, :, 1 + dy:1 + dy + STRIP]

            def Ctr(t):
                return t[:, :, 1:1 + STRIP]

            c2 = pairtmp.tile([128, R, STRIP], F32, tag="c2")
            s2 = pairtmp.tile([128, R, STRIP], F32, tag="s2")
            q = pairtmp.tile([128, R, STRIP], F32, tag="q")
            t1 = pairtmp.tile([128, R, STRIP], F32, tag="t1")
            nc.vector.tensor_tensor(c2, Ctr(c2c), S("c2c"), op=OP.add)
            nc.vector.tensor_tensor(s2, Ctr(s2c), S("s2c"), op=OP.add)
            nc.vector.tensor_tensor(q, c2, c2, op=OP.mult)
            nc.vector.tensor_tensor(t1, s2, s2, op=OP.mult)
            nc.vector.tensor_tensor(q, q, t1, op=OP.add)
            rin = pairtmp.tile([128, R, STRIP], F32, tag="rin")
            nc.scalar.activation(rin, q, AF.Ln)
            nc.scalar.activation(rin, rin, AF.Exp, scale=-0.5)
            nc.vector.tensor_scalar(rin, rin, 1e6, None, op0=OP.min)
            nc.vector.tensor_tensor(c2, c2, rin, op=OP.mult)
            nc.vector.tensor_tensor(s2, s2, rin, op=OP.mult)
            E = pairtmp.tile([128, R, STRIP], F32, tag="E")
            iE = pairtmp.tile([128, R, STRIP], F32, tag="iE")
            bp = pairtmp.tile([128, R, STRIP], F32, tag="bp")
            nc.vector.tensor_tensor(E, Ctr(Ft), S("Ft"), op=OP.mult)
            nc.vector.tensor_tensor(iE, Ctr(Gt), S("Gt"), op=OP.mult)
            nc.vector.tensor_tensor(bp, Ctr(bt), S("bt"), op=OP.add)
            rbp = pairtmp.tile([128, R, STRIP], F32, tag="rbp")
            nc.scalar.activation(rbp, bp, AF.Ln, bias=cb[:, 1:2])
            nc.scalar.activation(rbp, rbp, AF.Exp, scale=-2.0)
            pu2 = pairtmp.tile([128, R, STRIP], F32, tag="pu2")
            ps2 = pairtmp.tile([128, R, STRIP], F32, tag="ps2")
            a1, a2, a3 = dx * dx, dy * dy, dx * dy
            if a3 == 0:
                hc = 0.5 * (a1 - a2)
                nc.vector.tensor_scalar(pu2, c2, hc, 0.5, op0=OP.mult, op1=OP.add)
                nc.vector.tensor_scalar(ps2, c2, -hc, 0.5, op0=OP.mult, op1=OP.add)
            else:
                nc.vector.tensor_scalar(pu2, s2, float(a3), 1.0, op0=OP.mult, op1=OP.add)
                nc.vector.tensor_scalar(ps2, s2, float(-a3), 1.0, op0=OP.mult, op1=OP.add)
            nc.vector.tensor_tensor(pu2, pu2, iE, op=OP.mult)
            nc.vector.tensor_tensor(ps2, ps2, E, op=OP.mult)
            nc.vector.tensor_tensor(pu2, pu2, ps2, op=OP.add)
            nc.vector.tensor_tensor(pu2, pu2, rbp, op=OP.mult)
            kern = pairtmp.tile([128, R, STRIP], F32, tag="kern")
            nc.scalar.activation(kern, pu2, AF.Exp, scale=-4.0)

            # comu into full tensor; compat with shifted mu
            cm = comu[ip]
            nc.vector.tensor_tensor(cm[:, :, 1 + q0:1 + q0 + STRIP], kern,
                                    mu[:, :, 1 + q0:1 + q0 + STRIP], op=OP.mult)
            mu_sh = {0: mu, 1: mup, -1: mum}[dx]
            cp = pairtmp.tile([128, R, STRIP], F32, tag=f"cp{ip}")
            nc.vector.tensor_tensor(cp, kern,
                                    mu_sh[:, :, 1 + q0 + dy:1 + q0 + dy + STRIP],
                                    op=OP.mult)
            compat_t[ip] = cp
            nc.vector.tensor_tensor(den, den, cp, op=OP.add)
            # mirror compat = comu shifted by (-dy,-dx); window cols q0..q0+SW
            if dx != 0:
                ps = psum_sh.tile([128, R, SW], F32, tag="fsh")
                nc.tensor.matmul(ps, SHIFT_BF[-dx], cm[:, :, q0:q0 + SW],
                                 start=True, stop=True)
                cst = strip_pool.tile([128, R, SW], F32, tag=f"csh{ip}")
                nc.scalar.activation(cst, ps, AF.Copy)
                # local col for image row (rho - dy): (1 + q0 + i - dy) - q0
                mirror = cst[:, :, 1 - dy:1 - dy + STRIP]
            else:
                mirror = cm[:, :, 1 + q0 - dy:1 + q0 - dy + STRIP]
            comu_sh_t[ip] = mirror
            nc.vector.tensor_tensor(den, den, mirror, op=OP.add)

        if dbg is not None:
            nc.sync.dma_start(out=dbg["dbg_den"][:, k], in_=den)
        rden = strip_pool.tile([128, R, STRIP], F32, tag="rden")
        nc.scalar.activation(rden, den, AF.Ln, bias=cb[:, 2:3])
        nc.scalar.activation(rden, rden, AF.Exp, scale=-1.0)

        # normalized weights, packed bf16: Wt[g, dxi, dyi, r, rho]
        Wt = strip_pool.tile([128, 3, 3, R, STRIP], BF16, tag="Wt")
        DXI = {-1: 0, 0: 1, 1: 2}
        for ip, (dy, dx) in enumerate(PAIRS):
            w1 = pairtmp.tile([128, R, STRIP], F32, tag=f"w{ip}a")
            nc.vector.tensor_tensor(w1, compat_t[ip], rden, op=OP.mult)
            nc.gpsimd.tensor_copy(Wt[:, DXI[dx], dy + 1], w1)
            w2 = pairtmp.tile([128, R, STRIP], F32, tag=f"w{ip}b")
            nc.vector.tensor_tensor(w2, comu_sh_t[ip], rden, op=OP.mult)
            nc.gpsimd.tensor_copy(Wt[:, DXI[-dx], 1 - dy], w2)
        w0t = pairtmp.tile([128, R, STRIP], F32, tag="w0t")
        nc.vector.tensor_tensor(w0t, mu[:, :, 1 + q0:1 + q0 + STRIP], rden,
                                op=OP.mult)
        nc.gpsimd.tensor_copy(Wt[:, 1, 1], w0t)
        if dbg is not None:
            nc.sync.dma_start(out=dbg["dbg_wt"][:, k], in_=Wt)

        # partition-shift weights by -dx so products live on the z partition:
        # w'[g'] = w[g'-dx]; later C[g] = P[g+dx] via SHIFT[dx] matmul.
        Wsh = {0: Wt[:, 1]}
        for dx in (1, -1):
            ps = psum_sh.tile([128, 3, R, STRIP], F32, tag="wsh")
            nc.tensor.matmul(ps, SHIFT_BF[-dx], Wt[:, DXI[dx]],
                             start=True, stop=True)
            wshs = strip_pool.tile([128, 3, R, STRIP], BF16, tag=f"wsh{dx}")
            nc.scalar.activation(wshs, ps, AF.Copy)
            Wsh[dx] = wshs

        # ---- stencil: products on DVE (bf16 2x), 36-term sum on PE in PSUM ----
        for b in range(STRIP // BLK):
            rb = q0 + b * BLK
            rs = b * BLK
            P = {}
            for dx in (-1, 0, 1):
                Pt = ppool.tile([128, 3, R, Cout, BLK], BF16, tag=f"P{dx}")
                for dyi, dy in enumerate((-1, 0, 1)):
                    wb = Wsh[dx][:, dyi, :, None, rs:rs + BLK].to_broadcast(
                        [128, R, Cout, BLK])
                    nc.vector.tensor_tensor(
                        Pt[:, dyi],
                        zview[:, :, :, 1 + rb + dy:1 + rb + dy + BLK],
                        wb, op=OP.mult)
                P[dx] = Pt

            acc = psumB.tile([128, BLK, Cout], F32, tag="acc")
            n = 0
            for dx in (-1, 0, 1):
                for dyi in range(3):
                    for r in range(R):
                        rhs = P[dx][:, dyi, r].transpose([0, 2, 1])
                        nc.tensor.matmul(acc, SHIFT_BF[dx], rhs,
                                         start=(n == 0), stop=False,
                                         skip_group_check=True)
                        n += 1
            # + pw_b via rank-1 ones matmul (aux: ones lhsT, pwb_row rhs)
            nc.tensor.matmul(acc, aux_bf[:, 0:128], aux_bf[:, 128:640],
                             start=False, stop=True, skip_group_check=True)
            stg = ppool.tile([128, BLK, Cout], F32, tag="stg")
            nc.scalar.activation(stg, acc, AF.Copy)
            dst = out_d[rb * 128:(rb + BLK) * 128, :].rearrange(
                "(rho g) o -> g rho o", g=128)
            nc.sync.dma_start(out=dst, in_=stg)


def _host_prep(inputs):
    import ml_dtypes
    x = np.asarray(inputs["x"], np.float32)
    gate_w = np.asarray(inputs["gate_w"], np.float32)
    gate_b = np.asarray(inputs["gate_b"], np.float32)
    value_w = np.asarray(inputs["value_w"], np.float32)
    geom_w = np.asarray(inputs["geom_w"], np.float32)
    geom_b = np.asarray(inputs["geom_b"], np.float32)
    pw_w = np.asarray(inputs["pw_w"], np.float32)
    pw_b = np.asarray(inputs["pw_b"], np.float32)

    M = pw_w.reshape(Cout, R, C).transpose(1, 0, 2) @ value_w      # [R, Cout, C]
    wgq = np.concatenate([gate_w.T, geom_w.T], axis=1)             # [C, 16]
    wz = M.transpose(2, 0, 1).reshape(C, R * Cout)                 # [C, 256]

    bf = ml_dtypes.bfloat16
    w1 = wz          # [C, 256] -> bf16 (z projection)
    w2 = wgq         # [C, 16] f32 (gate/geom projection)

    smat = np.zeros((128, 384), np.float32)
    for g in range(128):   # Sp[k, g] = 1 iff k = g+1 ; Sm[k, g] = 1 iff k = g-1
        if g + 1 < 128:
            smat[g + 1, g] = 1.0
        if g - 1 >= 0:
            smat[g - 1, 128 + g] = 1.0
        smat[g, 256 + g] = 1.0
    smat_f32 = smat[:, 0:256].copy()

    aux = np.zeros((1, 640), np.float32)
    aux[0, 0:128] = 1.0
    aux[0, 128:640] = np.tile(pw_b[None, :], (BLK, 1)).reshape(-1)

    b_th, b_ba, b_hy = geom_b[0:4], geom_b[4:8], geom_b[8:12]
    cb_cols = np.zeros(40, np.float32)
    cb_cols[0:4] = gate_b
    cb_cols[4:8] = b_th
    cb_cols[8:12] = 2.0 * b_th
    cb_cols[12:16] = 2.0 * b_th + np.pi / 2
    cb_cols[16:20] = -np.pi / 2 - b_th
    cb_cols[20:24] = np.pi / 2 - b_th
    cb_cols[24:28] = -0.75 * np.pi - b_th
    cb_cols[28:32] = 0.25 * np.pi - b_th
    cb_cols[32:36] = b_ba
    cb_cols[36:40] = b_hy
    cbias = np.tile(cb_cols[None, :], (128, 1))

    xf = np.ascontiguousarray(x.reshape(B, C, L))

    return {
        "xf": xf,
        "xh": xf.astype(bf),
        "w1": w1.astype(bf),
        "w2": w2.astype(np.float32),
        "smat": smat_f32,
        "smatbf": smat.astype(bf),
        "aux": aux.astype(bf),
        "cbias": cbias,
    }


def make_in_maps(inputs):
    h = _host_prep(inputs)
    return [{"xf": h["xf"][b], "xh": h["xh"][b], "w1": h["w1"], "w2": h["w2"],
             "smat": h["smat"], "smatbf": h["smatbf"], "aux": h["aux"],
             "cbias": h["cbias"]} for b in range(B)]


def kernel(**inputs) -> np.ndarray:
    if "nc" not in _CACHE:
        _CACHE["nc"] = build_program()
    nc = _CACHE["nc"]
    in_maps = make_in_maps(inputs)
    res = run_bass_kernel_spmd(nc, in_maps, core_ids=list(range(NCORE)))
    out = np.stack([
        res.results[b]["out"].reshape(H, W, Cout).transpose(2, 0, 1)
        for b in range(B)
    ])
    return out.astype(np.float32)



# revision 16
# speedup vs baseline: 1.8742x; 1.8742x over previous
"""Trainium2 Bass kernel for nn_AZConv2d (fuzzy-rule hyperbolic-geometry message passing).

Self-contained: hardcodes shapes B=8,C=64,H=W=128,R=4,Cout=64; shards batch over 8 cores.

v3: Tensor-engine-friendly restructure of v2.
  - Phase A: per image row ONE bf16 stationary [x_hi(64); x_lo(64)] and three
    matmuls: z (N=256, from x_hi via fused pw*value weights), gq twice
    (N=16 each, rhs [w2h;w2l] then [w2l;w2h]) accumulating all four hi/lo
    cross terms in f32 PSUM -> fp32-grade gq from pure bf16 matmuls (no fp32
    LDWEIGHTS). gq biases (gate_b/geom_b) are folded into the PSUM->SBUF
    evacuation (strip-granular DVE add), so downstream field ops need no
    per-rule bias and run 4-rules-wide with immediate constants.
  - z: [128, 256, 130] fp16 (rho innermost; halo cols zero). z evac in 2-row
    PSUM tiles (one copy per 2 rows, alternating ACT/DVE).
  - Fields: full-image [128, 4, 130] ops (4x fewer, 4x bigger than v2's
    strips). Sin ops grouped before Exp/Ln ops to minimize ACT table swaps.
  - Stencil: products on DVE (fp16 2x) into P[dx][128, 3dy, 4r, 64o, 8rho]
    with rho innermost everywhere; the 36-term (dx,dy,r) sum runs on the PE
    as PSUM-accumulating shift matmuls whose rhs is CONTIGUOUS (N=512) and
    whose dst is a transposed view of acc[128, 8, 64] - v2 streamed a
    transposed rhs which ran at ~2.1ns/row; contiguous streams run ~4x faster.
  - All shift matrices fp16 for fp16 streams (field shifts stay f32).
"""
import numpy as np
from contextlib import ExitStack

import concourse.bass as bass
import concourse.tile as tile
from concourse import mybir
from concourse.bass_utils import run_bass_kernel_spmd

F32 = mybir.dt.float32
F16 = mybir.dt.float16
BF16 = mybir.dt.bfloat16
AF = mybir.ActivationFunctionType
OP = mybir.AluOpType

B, C, H, W, R, Cout = 8, 64, 128, 128, 4, 64
L = H * W
NCORE = 8
BLK = 8                 # stencil rows per psum accumulation block
NBLK = H // BLK         # 16
SA = 32                 # phase A strip rows
PI = float(np.pi)
PAIRS = [(0, 1), (1, -1), (1, 0), (1, 1)]   # (dy, dx)
DXI = {-1: 0, 0: 1, 1: 2}

_CACHE = {}


def split_multiwaits(nc):
    """This walrus accepts ONE sync wait per instruction: split extras into
    same-engine NoOps inserted just before the instruction."""
    n = 0
    for bb in nc.main_func.blocks:
        out = []
        for ins in bb.instructions:
            si = ins.sync_info
            if si is not None and len(si.on_wait) > 1:
                waits = list(si.on_wait)
                for w in waits[:-1]:
                    n += 1
                    nop = mybir.InstNoOp(name=f"WSPLIT-{n}")
                    nop.engine = ins.engine
                    nop.sync_info = mybir.SyncInfo(on_wait=[w], on_update=[])
                    out.append(nop)
                ins.sync_info = mybir.SyncInfo(on_wait=[waits[-1]],
                                               on_update=list(si.on_update))
            out.append(ins)
        bb.instructions[:] = out
    return n


def build_program(debug=False):
    nc = bass.Bass()
    xslab_d = nc.dram_tensor("xslab", [128, L], BF16, kind="ExternalInput")
    x2_d = nc.dram_tensor("x2slab", [64, L], BF16, kind="ExternalInput")
    wA_d = nc.dram_tensor("wA", [128, 304], BF16, kind="ExternalInput")
    smat_d = nc.dram_tensor("smat", [128, 256], F32, kind="ExternalInput")
    smath_d = nc.dram_tensor("smath", [128, 384], F16, kind="ExternalInput")
    gqb_d = nc.dram_tensor("gqbias", [128, 16], F32, kind="ExternalInput")
    aux_d = nc.dram_tensor("aux", [1, 640], BF16, kind="ExternalInput")
    out_d = nc.dram_tensor("out", [L, Cout], F32, kind="ExternalOutput")
    dbg = None
    if debug:
        dbg = {
            "dbg_gq": nc.dram_tensor("dbg_gq", [128, 16, H + 2], F32,
                                     kind="ExternalOutput")[:],
            "dbg_z": nc.dram_tensor("dbg_z", [128, 256, H + 2], F16,
                                    kind="ExternalOutput")[:],
            "dbg_mu": nc.dram_tensor("dbg_mu", [128, R, H + 2], F32,
                                     kind="ExternalOutput")[:],
            "dbg_wt": nc.dram_tensor("dbg_wt", [128, 3, 3, R, H + 2], F16,
                                     kind="ExternalOutput")[:],
            "dbg_den": nc.dram_tensor("dbg_den", [128, R, H], F32,
                                      kind="ExternalOutput")[:],
        }

    with ExitStack() as ctx:
        tc = ctx.enter_context(tile.TileContext(nc))
        _emit(ctx, tc, xslab_d[:], x2_d[:], wA_d[:], smat_d[:], smath_d[:],
              gqb_d[:], aux_d[:], out_d[:], dbg)
    split_multiwaits(nc)
    return nc


def _emit(ctx, tc, xslab_d, x2_d, wA_d, smat_d, smath_d, gqb_d, aux_d, out_d,
          dbg=None):
    nc = tc.nc

    persist = ctx.enter_context(tc.tile_pool(name="persist", bufs=1))

    # ---------------- persistent tensors ----------------
    wA_sb = persist.tile([128, 304], BF16)
    nc.sync.dma_start(out=wA_sb, in_=wA_d)
    smat = persist.tile([128, 256], F32)       # [Sp | Sm] f32
    nc.sync.dma_start(out=smat, in_=smat_d)
    smath = persist.tile([128, 384], F16)      # [Sp | Sm | I] fp16
    nc.sync.dma_start(out=smath, in_=smath_d)
    gqbias = persist.tile([128, 16], F32)
    nc.sync.dma_start(out=gqbias, in_=gqb_d)
    aux = persist.tile([1, 640], BF16)         # [ones(128) | pwb_row(512)]
    nc.sync.dma_start(out=aux, in_=aux_d)

    # bias constants for ACT ops ([P,1] APs)
    cb = persist.tile([128, 4], F32)
    nc.vector.memset(cb[:, 0:1], 1e-30)
    nc.vector.memset(cb[:, 1:2], 2e-4)
    nc.vector.memset(cb[:, 2:3], 1e-6)
    nc.vector.memset(cb[:, 3:4], float(np.pi / 2))

    SHIFT = {1: smat[:, 0:128], -1: smat[:, 128:256]}
    SHIFTH = {1: smath[:, 0:128], -1: smath[:, 128:256], 0: smath[:, 256:384]}

    # z: [128, 256 ch, 130 rho] fp16, rho innermost, zero halo cols.
    zbuf = persist.tile([128, 256, H + 2], F16)
    nc.vector.memset(zbuf[:, :, 0], 0.0)
    nc.vector.memset(zbuf[:, :, H + 1], 0.0)
    zview = zbuf.rearrange("p (r o) c -> p r o c", r=R)
    # gq: [128, 16 fields, 130] f32 (biases pre-added on evac; halo = 0)
    gq = persist.tile([128, 16, H + 2], F32)
    nc.vector.memset(gq[:, :, 0], 0.0)
    nc.vector.memset(gq[:, :, H + 1], 0.0)

    # normalized weights Wt[g, dxi, dyi, r, rho] fp16 + partition-shifted WQ
    Wt = persist.tile([128, 3, 3, R, H + 2], F16)
    WQp = persist.tile([128, 3, R, H + 2], F16)   # dx=+1 group shifted by -1
    WQm = persist.tile([128, 3, R, H + 2], F16)   # dx=-1 group shifted by +1
    WQ = {1: WQp, -1: WQm, 0: Wt[:, 1]}

    # ---------------- phase A ----------------
    with tc.tile_pool(name="phA", bufs=2) as pha, \
         tc.tile_pool(name="psA", bufs=2, space="PSUM") as psA, \
         tc.tile_pool(name="psG", bufs=2, space="PSUM") as psG:
        for k in range(H // SA):
            q0 = k * SA
            xw = pha.tile([128, SA * 128], BF16, tag="xw")
            nc.sync.dma_start(out=xw, in_=xslab_d[:, q0 * 128:(q0 + SA) * 128])
            xw2 = pha.tile([64, SA * 128], BF16, tag="xw2")
            nc.sync.dma_start(out=xw2, in_=x2_d[:, q0 * 128:(q0 + SA) * 128])
            gqp = psG.tile([128, SA * 16], F32, tag="gqp")
            for j2 in range(SA // 2):
                j = 2 * j2
                pt2 = psA.tile([128, 2, 256], F32, tag="pt2")
                for i in (0, 1):
                    lhsT = xw[:, (j + i) * 128:(j + i + 1) * 128]
                    nc.tensor.matmul(pt2[:, i], lhsT, wA_sb[:, 0:256],
                                     start=True, stop=True,
                                     skip_group_check=True)
                    g16 = gqp[:, (j + i) * 16:(j + i + 1) * 16]
                    # gq = (x0+x1+x2)^T (w0+w1+w2) to ~fp32 accuracy:
                    # [w0;w1]+[w1;w0]+[w2;w2] on [x0;x1], then x2^T w0.
                    nc.tensor.matmul(g16, lhsT, wA_sb[:, 256:272],
                                     start=True, stop=False,
                                     skip_group_check=True)
                    nc.tensor.matmul(g16, lhsT, wA_sb[:, 272:288],
                                     start=False, stop=False,
                                     skip_group_check=True)
                    nc.tensor.matmul(g16, lhsT, wA_sb[:, 288:304],
                                     start=False, stop=False,
                                     skip_group_check=True)
                    nc.tensor.matmul(g16, xw2[:, (j + i) * 128:(j + i + 1) * 128],
                                     wA_sb[0:64, 256:272],
                                     start=False, stop=True,
                                     skip_group_check=True)
                dst = zbuf[:, :, 1 + q0 + j:3 + q0 + j]   # [128, 256, 2]
                src = pt2.transpose([0, 2, 1])            # [128, 256, 2]
                if j2 % 2 == 0:
                    nc.scalar.activation(dst, src, AF.Copy)
                else:
                    nc.vector.tensor_copy(dst, src)
            # gq strip evac with bias add: psum [32, 16] -> gq [16, 32]
            dstg = gq[:, :, 1 + q0:1 + q0 + SA]
            srcg = gqp.rearrange("p (j c) -> p c j", c=16)
            bcol = gqbias[:, :, None].to_broadcast([128, 16, SA])
            nc.vector.tensor_tensor(dstg, srcg, bcol, op=OP.add)

    if dbg is not None:
        nc.sync.dma_start(out=dbg["dbg_gq"], in_=gq)
        nc.sync.dma_start(out=dbg["dbg_z"], in_=zbuf)

    # ---------------- fields (full image) ----------------
    with tc.tile_pool(name="phF", bufs=1) as fld, \
         tc.tile_pool(name="psF", bufs=2, space="PSUM") as psF, \
         tc.tile_pool(name="fsetup", bufs=1) as fst, \
         tc.tile_pool(name="ptmp", bufs=1) as ptp:

        def shift_into(dst_t, src_ap, sgn, dtype_f32, nch):
            """dst[g] = src[g+sgn]; src/dst [128, nch, 130]; 2-rule chunks."""
            step = 2
            for c0 in range(0, nch, step):
                ps = psF.tile([128, step, H + 2], F32, tag="psh")
                if dtype_f32:
                    nc.tensor.matmul(ps, SHIFT[sgn], src_ap[:, c0:c0 + step],
                                     start=True, stop=True,
                                     skip_group_check=True)
                else:
                    nc.tensor.matmul(ps, SHIFTH[sgn], src_ap[:, c0:c0 + step],
                                     start=True, stop=True,
                                     skip_group_check=True)
                if (c0 // step) % 2 == 0:
                    nc.scalar.activation(dst_t[:, c0:c0 + step], ps, AF.Copy)
                else:
                    nc.vector.tensor_copy(dst_t[:, c0:c0 + step], ps)

        thw = gq[:, 4:8, :]     # theta + b_th
        # --- theta path first (Sin table) ---
        m1 = fst.tile([128, R, H + 2], F32, tag="m1")
        m2 = fst.tile([128, R, H + 2], F32, tag="m2")
        tred = fst.tile([128, R, H + 2], F32, tag="tred")
        tred2 = fst.tile([128, R, H + 2], F32, tag="tred2")
        s2cF = fld.tile([128, R, H + 2], F32, tag="s2cF")
        c2cF = fld.tile([128, R, H + 2], F32, tag="c2cF")
        nc.vector.tensor_scalar(m1, thw, -PI / 2, None, op0=OP.is_lt)
        nc.vector.tensor_scalar(m2, thw, PI / 2, None, op0=OP.is_gt)
        nc.vector.tensor_tensor(m1, m1, m2, op=OP.subtract)
        nc.vector.scalar_tensor_tensor(out=tred, in0=m1, scalar=PI, in1=thw,
                                       op0=OP.mult, op1=OP.add)
        nc.scalar.activation(s2cF, tred, AF.Sin, scale=2.0)
        nc.vector.tensor_scalar(m1, thw, -0.75 * PI, None, op0=OP.is_lt)
        nc.vector.tensor_scalar(m2, thw, 0.25 * PI, None, op0=OP.is_gt)
        nc.vector.tensor_tensor(m1, m1, m2, op=OP.subtract)
        nc.vector.scalar_tensor_tensor(out=tred2, in0=m1, scalar=PI, in1=thw,
                                       op0=OP.mult, op1=OP.add)
        nc.scalar.activation(c2cF, tred2, AF.Sin, scale=2.0, bias=cb[:, 3:4])

        # --- softmax mu (Exp/Ln table) ---
        eg = fst.tile([128, R, H + 2], F32, tag="eg")
        nc.scalar.activation(eg, gq[:, 0:4, :], AF.Exp)
        nc.vector.memset(eg[:, :, 0], 0.0)
        nc.vector.memset(eg[:, :, H + 1], 0.0)
        zsum = fst.tile([128, H + 2], F32, tag="zsum")
        nc.vector.tensor_tensor(zsum, eg[:, 0], eg[:, 1], op=OP.add)
        nc.vector.tensor_tensor(zsum, zsum, eg[:, 2], op=OP.add)
        nc.vector.tensor_tensor(zsum, zsum, eg[:, 3], op=OP.add)
        rz = fst.tile([128, H + 2], F32, tag="rz")
        nc.scalar.activation(rz, zsum, AF.Ln, bias=cb[:, 0:1])
        nc.scalar.activation(rz, rz, AF.Exp, scale=-1.0)
        mu = fld.tile([128, R, H + 2], F32, tag="mu")
        rzb = rz[:, None, :].to_broadcast([128, R, H + 2])
        nc.vector.tensor_tensor(mu, eg, rzb, op=OP.mult)

        # --- hyper / base fields ---
        uh = fst.tile([128, R, H + 2], F32, tag="uh")
        nc.scalar.activation(uh, gq[:, 12:16, :], AF.Exp)
        ub = fst.tile([128, R, H + 2], F32, tag="ub")
        nc.scalar.activation(ub, gq[:, 8:12, :], AF.Exp)
        Ft = fld.tile([128, R, H + 2], F32, tag="Ft")
        nc.vector.tensor_scalar_add(Ft, uh, 1.0)
        lnf = fst.tile([128, R, H + 2], F32, tag="lnf")
        nc.scalar.activation(lnf, uh, AF.Ln, bias=1.0)
        Gt = fld.tile([128, R, H + 2], F32, tag="Gt")
        nc.scalar.activation(Gt, lnf, AF.Exp, scale=-1.0)
        bt = fld.tile([128, R, H + 2], F32, tag="bt")
        nc.scalar.activation(bt, ub, AF.Ln, bias=1.0)

        if dbg is not None:
            nc.sync.dma_start(out=dbg["dbg_mu"], in_=mu)

        # --- shifted copies ---
        base = {"c2c": c2cF, "s2c": s2cF, "Ft": Ft, "Gt": Gt, "bt": bt}
        shifted = {}
        for name, t in base.items():
            d = {0: t}
            for sgn in (1, -1):
                st = fld.tile([128, R, H + 2], F32, tag=f"{name}s{sgn}")
                shift_into(st, t, sgn, True, R)
                d[sgn] = st
            shifted[name] = d
        mu16 = fld.tile([128, R, H + 2], F16, tag="mu16")
        nc.vector.tensor_copy(mu16, mu)
        mup = fld.tile([128, R, H + 2], F16, tag="mup")
        mum = fld.tile([128, R, H + 2], F16, tag="mum")
        shift_into(mup, mu16, 1, False, R)
        shift_into(mum, mu16, -1, False, R)
        MUSH = {0: mu16, 1: mup, -1: mum}

        # --- pair loop ---
        comu = [fld.tile([128, R, H + 2], F16, tag=f"comu{i}",
                         name=f"comu{i}") for i in range(4)]
        for cm in comu:
            nc.vector.memset(cm[:, :, 0], 0.0)
            nc.vector.memset(cm[:, :, H + 1], 0.0)
        den = fld.tile([128, R, H], F32, tag="den")
        compat_t = {}
        mirror_t = {}

        def Ctr(t):
            return t[:, :, 1:1 + H]

        for ip, (dy, dx) in enumerate(PAIRS):
            def S(name):
                return shifted[name][dx][:, :, 1 + dy:1 + dy + H]

            c2 = ptp.tile([128, R, H], F32, tag="c2")
            s2 = ptp.tile([128, R, H], F32, tag="s2")
            q = ptp.tile([128, R, H], F32, tag="q")
            t1 = ptp.tile([128, R, H], F32, tag="t1")
            nc.vector.tensor_tensor(c2, Ctr(c2cF), S("c2c"), op=OP.add)
            nc.vector.tensor_tensor(s2, Ctr(s2cF), S("s2c"), op=OP.add)
            nc.vector.tensor_tensor(q, c2, c2, op=OP.mult)
            nc.vector.tensor_tensor(t1, s2, s2, op=OP.mult)
            nc.vector.tensor_tensor(q, q, t1, op=OP.add)
            rin = ptp.tile([128, R, H], F32, tag="rin")
            nc.scalar.activation(rin, q, AF.Ln)
            nc.scalar.activation(rin, rin, AF.Exp, scale=-0.5)
            nc.vector.tensor_scalar(rin, rin, 1e6, None, op0=OP.min)
            nc.vector.tensor_tensor(c2, c2, rin, op=OP.mult)
            nc.vector.tensor_tensor(s2, s2, rin, op=OP.mult)
            E = ptp.tile([128, R, H], F32, tag="E")
            iE = ptp.tile([128, R, H], F32, tag="iE")
            bp = ptp.tile([128, R, H], F32, tag="bp")
            nc.vector.tensor_tensor(E, Ctr(Ft), S("Ft"), op=OP.mult)
            nc.vector.tensor_tensor(iE, Ctr(Gt), S("Gt"), op=OP.mult)
            nc.vector.tensor_tensor(bp, Ctr(bt), S("bt"), op=OP.add)
            rbp = ptp.tile([128, R, H], F32, tag="rbp")
            nc.scalar.activation(rbp, bp, AF.Ln, bias=cb[:, 1:2])
            nc.scalar.activation(rbp, rbp, AF.Exp, scale=-2.0)
            pu2 = ptp.tile([128, R, H], F32, tag="pu2")
            ps2 = ptp.tile([128, R, H], F32, tag="ps2")
            a1, a2, a3 = dx * dx, dy * dy, dx * dy
            if a3 == 0:
                hc = 0.5 * (a1 - a2)
                nc.vector.tensor_scalar(pu2, c2, hc, 0.5, op0=OP.mult,
                                        op1=OP.add)
                nc.vector.tensor_scalar(ps2, c2, -hc, 0.5, op0=OP.mult,
                                        op1=OP.add)
            else:
                nc.vector.tensor_scalar(pu2, s2, float(a3), 1.0, op0=OP.mult,
                                        op1=OP.add)
                nc.vector.tensor_scalar(ps2, s2, float(-a3), 1.0, op0=OP.mult,
                                        op1=OP.add)
            nc.vector.tensor_tensor(pu2, pu2, iE, op=OP.mult)
            nc.vector.tensor_tensor(ps2, ps2, E, op=OP.mult)
            nc.vector.tensor_tensor(pu2, pu2, ps2, op=OP.add)
            nc.vector.tensor_tensor(pu2, pu2, rbp, op=OP.mult)
            kern = ptp.tile([128, R, H], F32, tag="kern")
            nc.scalar.activation(kern, pu2, AF.Exp, scale=-4.0)

            nc.vector.tensor_tensor(comu[ip][:, :, 1:1 + H], kern, Ctr(mu),
                                    op=OP.mult)
            cp = fld.tile([128, R, H], F32, tag=f"cp{ip}")
            nc.vector.tensor_tensor(
                cp, kern, MUSH[dx][:, :, 1 + dy:1 + dy + H], op=OP.mult)
            compat_t[ip] = cp
            if ip == 0:
                nc.vector.tensor_tensor(den, Ctr(mu), cp, op=OP.add)
            else:
                nc.vector.tensor_tensor(den, den, cp, op=OP.add)
            # mirror compat = comu shifted by (-dy, -dx)
            if dx != 0:
                cst = fld.tile([128, R, H + 2], F16, tag=f"csh{ip}")
                shift_into(cst, comu[ip], -dx, False, R)
                mirror = cst[:, :, 1 - dy:1 - dy + H]
            else:
                mirror = comu[ip][:, :, 1 - dy:1 - dy + H]
            mirror_t[ip] = mirror
            nc.vector.tensor_tensor(den, den, mirror, op=OP.add)

        if dbg is not None:
            nc.sync.dma_start(out=dbg["dbg_den"], in_=den)
        rden = fld.tile([128, R, H], F32, tag="rden")
        nc.scalar.activation(rden, den, AF.Ln, bias=cb[:, 2:3])
        nc.scalar.activation(rden, rden, AF.Exp, scale=-1.0)

        # --- normalized weights into Wt ---
        for ip, (dy, dx) in enumerate(PAIRS):
            nc.vector.tensor_tensor(Wt[:, DXI[dx], 1 + dy, :, 1:1 + H],
                                    compat_t[ip], rden, op=OP.mult)
            nc.vector.tensor_tensor(Wt[:, DXI[-dx], 1 - dy, :, 1:1 + H],
                                    mirror_t[ip], rden, op=OP.mult)
        nc.vector.tensor_tensor(Wt[:, 1, 1, :, 1:1 + H], Ctr(mu), rden,
                                op=OP.mult)
        if dbg is not None:
            nc.sync.dma_start(out=dbg["dbg_wt"], in_=Wt)

        # --- partition-shift dx groups: WQ[dx][g] = Wt[dx-group][g-dx] ---
        for dx, wq in ((1, WQp), (-1, WQm)):
            src = Wt[:, DXI[dx]].rearrange("p a r c -> p (a r) c")
            dst = wq.rearrange("p a r c -> p (a r) c")
            shift_into(dst, src, -dx, False, 3 * R)

    # ---------------- stencil ----------------
    with tc.tile_pool(name="phC", bufs=2) as pc, \
         tc.tile_pool(name="psC", bufs=2, space="PSUM") as psC:
        for b in range(NBLK):
            r0 = b * BLK
            P = {}
            for dx in (-1, 0, 1):
                Pt = pc.tile([128, 3, R, Cout, BLK], F16, tag=f"P{dx}")
                for dyi, dy in enumerate((-1, 0, 1)):
                    wb = WQ[dx][:, dyi, :, None,
                                1 + r0:1 + r0 + BLK].to_broadcast(
                        [128, R, Cout, BLK])
                    nc.vector.tensor_tensor(
                        Pt[:, dyi],
                        zview[:, :, :, r0 + dy + 1:r0 + dy + 1 + BLK],
                        wb, op=OP.mult)
                P[dx] = Pt

            acc = psC.tile([128, BLK, Cout], F32, tag="acc")
            accT = acc.transpose([0, 2, 1])   # [128, 64, 8] view
            n = 0
            for dx in (-1, 0, 1):
                for dyi in range(3):
                    for r in range(R):
                        rhs = P[dx][:, dyi, r]       # [128, 64, 8] contiguous
                        nc.tensor.matmul(accT, SHIFTH[dx], rhs,
                                         start=(n == 0), stop=False,
                                         skip_group_check=True)
                        n += 1
            nc.tensor.matmul(acc, aux[:, 0:128], aux[:, 128:640],
                             start=False, stop=True, skip_group_check=True)
            stg = pc.tile([128, BLK, Cout], F32, tag="stg")
            nc.scalar.activation(stg, acc, AF.Copy)
            dst = out_d[r0 * 128:(r0 + BLK) * 128, :].rearrange(
                "(rho g) o -> g rho o", g=128)
            nc.sync.dma_start(out=dst, in_=stg)


def _host_prep(inputs):
    import ml_dtypes
    bf = ml_dtypes.bfloat16
    x = np.asarray(inputs["x"], np.float32)
    gate_w = np.asarray(inputs["gate_w"], np.float32)
    gate_b = np.asarray(inputs["gate_b"], np.float32)
    value_w = np.asarray(inputs["value_w"], np.float32)
    geom_w = np.asarray(inputs["geom_w"], np.float32)
    geom_b = np.asarray(inputs["geom_b"], np.float32)
    pw_w = np.asarray(inputs["pw_w"], np.float32)
    pw_b = np.asarray(inputs["pw_b"], np.float32)

    M = pw_w.reshape(Cout, R, C).transpose(1, 0, 2) @ value_w      # [R,Cout,C]
    wz = M.transpose(2, 0, 1).reshape(C, R * Cout)                 # [C, 256]
    wgq = np.concatenate([gate_w.T, geom_w.T], axis=1)             # [C, 16]

    xf = np.ascontiguousarray(x.reshape(B, C, L))
    xh = xf.astype(bf)
    xl = (xf - xh.astype(np.float32)).astype(bf)
    x2 = (xf - xh.astype(np.float32) - xl.astype(np.float32)).astype(bf)
    xslab = np.concatenate([xh, xl], axis=1)                       # [B,128,L]

    w2h = wgq.astype(bf)
    w2l = (wgq - w2h.astype(np.float32)).astype(bf)
    w2q = (wgq - w2h.astype(np.float32) - w2l.astype(np.float32)).astype(bf)
    wA = np.zeros((128, 304), np.float32)
    wA[0:64, 0:256] = wz
    wA[0:64, 256:272] = w2h.astype(np.float32)
    wA[64:128, 256:272] = w2l.astype(np.float32)
    wA[0:64, 272:288] = w2l.astype(np.float32)
    wA[64:128, 272:288] = w2h.astype(np.float32)
    wA[0:64, 288:304] = w2q.astype(np.float32)
    wA[64:128, 288:304] = w2q.astype(np.float32)

    smat = np.zeros((128, 384), np.float32)
    for g in range(128):   # Sp[k, g] = 1 iff k = g+1 ; Sm[k, g] = 1 iff k=g-1
        if g + 1 < 128:
            smat[g + 1, g] = 1.0
        if g - 1 >= 0:
            smat[g - 1, 128 + g] = 1.0
        smat[g, 256 + g] = 1.0

    aux = np.zeros((1, 640), np.float32)
    aux[0, 0:128] = 1.0
    aux[0, 128:640] = np.tile(pw_b[None, :], (BLK, 1)).reshape(-1)

    gqb_cols = np.concatenate([gate_b, geom_b])                    # [16]
    gqbias = np.tile(gqb_cols[None, :], (128, 1)).astype(np.float32)

    return {
        "xslab": xslab,
        "x2slab": x2,
        "wA": wA.astype(bf),
        "smat": smat[:, 0:256].copy(),
        "smath": smat.astype(np.float16),
        "gqbias": gqbias,
        "aux": aux.astype(bf),
    }


def make_in_maps(inputs):
    h = _host_prep(inputs)
    return [{"xslab": h["xslab"][b], "x2slab": h["x2slab"][b], "wA": h["wA"],
             "smat": h["smat"], "smath": h["smath"], "gqbias": h["gqbias"],
             "aux": h["aux"]} for b in range(B)]


def kernel(**inputs) -> np.ndarray:
    if "nc" not in _CACHE:
        _CACHE["nc"] = build_program()
    nc = _CACHE["nc"]
    in_maps = make_in_maps(inputs)
    res = run_bass_kernel_spmd(nc, in_maps, core_ids=list(range(NCORE)))
    out = np.stack([
        res.results[b]["out"].reshape(H, W, Cout).transpose(2, 0, 1)
        for b in range(B)
    ])
    return out.astype(np.float32)


# revision 18
# speedup vs baseline: 1.8771x; 1.0016x over previous
"""Trainium2 Bass kernel for nn_AZConv2d (fuzzy-rule hyperbolic-geometry message passing).

Self-contained: hardcodes shapes B=8,C=64,H=W=128,R=4,Cout=64; shards batch over 8 cores.

v3: Tensor-engine-friendly restructure of v2.
  - Phase A: per image row ONE bf16 stationary [x_hi(64); x_lo(64)] and three
    matmuls: z (N=256, from x_hi via fused pw*value weights), gq twice
    (N=16 each, rhs [w2h;w2l] then [w2l;w2h]) accumulating all four hi/lo
    cross terms in f32 PSUM -> fp32-grade gq from pure bf16 matmuls (no fp32
    LDWEIGHTS). gq biases (gate_b/geom_b) are folded into the PSUM->SBUF
    evacuation (strip-granular DVE add), so downstream field ops need no
    per-rule bias and run 4-rules-wide with immediate constants.
  - z: [128, 256, 130] fp16 (rho innermost; halo cols zero). z evac in 2-row
    PSUM tiles (one copy per 2 rows, alternating ACT/DVE).
  - Fields: full-image [128, 4, 130] ops (4x fewer, 4x bigger than v2's
    strips). Sin ops grouped before Exp/Ln ops to minimize ACT table swaps.
  - Stencil: products on DVE (fp16 2x) into P[dx][128, 3dy, 4r, 64o, 8rho]
    with rho innermost everywhere; the 36-term (dx,dy,r) sum runs on the PE
    as PSUM-accumulating shift matmuls whose rhs is CONTIGUOUS (N=512) and
    whose dst is a transposed view of acc[128, 8, 64] - v2 streamed a
    transposed rhs which ran at ~2.1ns/row; contiguous streams run ~4x faster.
  - All shift matrices fp16 for fp16 streams (field shifts stay f32).
"""
import numpy as np
from contextlib import ExitStack

import concourse.bass as bass
import concourse.tile as tile
from concourse import mybir
from concourse.bass_utils import run_bass_kernel_spmd

F32 = mybir.dt.float32
F16 = mybir.dt.float16
BF16 = mybir.dt.bfloat16
AF = mybir.ActivationFunctionType
OP = mybir.AluOpType

B, C, H, W, R, Cout = 8, 64, 128, 128, 4, 64
L = H * W
NCORE = 8
BLK = 8                 # stencil rows per psum accumulation block
NBLK = H // BLK         # 16
SA = 32                 # phase A strip rows
PI = float(np.pi)
PAIRS = [(0, 1), (1, -1), (1, 0), (1, 1)]   # (dy, dx)
DXI = {-1: 0, 0: 1, 1: 2}

_CACHE = {}


def split_multiwaits(nc):
    """This walrus accepts ONE sync wait per instruction: split extras into
    same-engine NoOps inserted just before the instruction."""
    n = 0
    for bb in nc.main_func.blocks:
        out = []
        for ins in bb.instructions:
            si = ins.sync_info
            if si is not None and len(si.on_wait) > 1:
                waits = list(si.on_wait)
                for w in waits[:-1]:
                    n += 1
                    nop = mybir.InstNoOp(name=f"WSPLIT-{n}")
                    nop.engine = ins.engine
                    nop.sync_info = mybir.SyncInfo(on_wait=[w], on_update=[])
                    out.append(nop)
                ins.sync_info = mybir.SyncInfo(on_wait=[waits[-1]],
                                               on_update=list(si.on_update))
            out.append(ins)
        bb.instructions[:] = out
    return n


def dedupe_ldweights(nc):
    """The tile scheduler emits one LDWEIGHTS per matmul even when many
    consecutive matmuls stream against the identical stationary (e.g. the 12
    shift matmuls per dx group). A reload of the already-loaded array costs
    ~215ns and serializes with the stream. Convert LDWEIGHTS whose weight AP
    (and tile cfg) matches the previous PE weight load into NoOps, keeping
    sync_info so semaphore semantics are unchanged."""
    n = 0
    for bb in nc.main_func.blocks:
        last_sig = None
        out = []
        for ins in bb.instructions:
            tn = type(ins).__name__
            if tn == 'InstLdweights':
                sig = (str(ins.ins[0]),
                       str(getattr(ins, 'tile_position', None)),
                       str(getattr(ins, 'tile_size', None)),
                       str(getattr(ins, 'perf_mode', None)),
                       str(getattr(ins, 'is_transpose', None)))
                if sig == last_sig:
                    n += 1
                    nop = mybir.InstNoOp(name=f"LWDEDUP-{n}")
                    nop.engine = ins.engine
                    nop.sync_info = ins.sync_info
                    out.append(nop)
                    continue
                last_sig = sig
            out.append(ins)
        bb.instructions[:] = out
    return n


def build_program(debug=False):
    nc = bass.Bass()
    xslab_d = nc.dram_tensor("xslab", [128, L], BF16, kind="ExternalInput")
    x2_d = nc.dram_tensor("x2slab", [64, L], BF16, kind="ExternalInput")
    wA_d = nc.dram_tensor("wA", [128, 304], BF16, kind="ExternalInput")
    smat_d = nc.dram_tensor("smat", [128, 256], F32, kind="ExternalInput")
    smath_d = nc.dram_tensor("smath", [128, 384], F16, kind="ExternalInput")
    gqb_d = nc.dram_tensor("gqbias", [128, 16], F32, kind="ExternalInput")
    aux_d = nc.dram_tensor("aux", [1, 640], BF16, kind="ExternalInput")
    out_d = nc.dram_tensor("out", [L, Cout], F32, kind="ExternalOutput")
    dbg = None
    if debug:
        dbg = {
            "dbg_gq": nc.dram_tensor("dbg_gq", [128, 16, H + 2], F32,
                                     kind="ExternalOutput")[:],
            "dbg_z": nc.dram_tensor("dbg_z", [128, 256, H + 2], F16,
                                    kind="ExternalOutput")[:],
            "dbg_mu": nc.dram_tensor("dbg_mu", [128, R, H + 2], F32,
                                     kind="ExternalOutput")[:],
            "dbg_wt": nc.dram_tensor("dbg_wt", [128, 3, 3, R, H + 2], F16,
                                     kind="ExternalOutput")[:],
            "dbg_den": nc.dram_tensor("dbg_den", [128, R, H], F32,
                                      kind="ExternalOutput")[:],
        }

    with ExitStack() as ctx:
        tc = ctx.enter_context(tile.TileContext(nc))
        _emit(ctx, tc, xslab_d[:], x2_d[:], wA_d[:], smat_d[:], smath_d[:],
              gqb_d[:], aux_d[:], out_d[:], dbg)
    ndup = dedupe_ldweights(nc)
    split_multiwaits(nc)
    if ndup == 0:
        log_msg = "dedupe_ldweights removed nothing"
    return nc


def _emit(ctx, tc, xslab_d, x2_d, wA_d, smat_d, smath_d, gqb_d, aux_d, out_d,
          dbg=None):
    nc = tc.nc

    persist = ctx.enter_context(tc.tile_pool(name="persist", bufs=1))

    # ---------------- persistent tensors ----------------
    wA_sb = persist.tile([128, 304], BF16)
    nc.sync.dma_start(out=wA_sb, in_=wA_d)
    smat = persist.tile([128, 256], F32)       # [Sp | Sm] f32
    nc.sync.dma_start(out=smat, in_=smat_d)
    smath = persist.tile([128, 384], F16)      # [Sp | Sm | I] fp16
    nc.sync.dma_start(out=smath, in_=smath_d)
    gqbias = persist.tile([128, 16], F32)
    nc.sync.dma_start(out=gqbias, in_=gqb_d)
    aux = persist.tile([1, 640], BF16)         # [ones(128) | pwb_row(512)]
    nc.sync.dma_start(out=aux, in_=aux_d)

    # bias constants for ACT ops ([P,1] APs)
    cb = persist.tile([128, 4], F32)
    nc.vector.memset(cb[:, 0:1], 1e-30)
    nc.vector.memset(cb[:, 1:2], 2e-4)
    nc.vector.memset(cb[:, 2:3], 1e-6)
    nc.vector.memset(cb[:, 3:4], float(np.pi / 2))

    SHIFT = {1: smat[:, 0:128], -1: smat[:, 128:256]}
    SHIFTH = {1: smath[:, 0:128], -1: smath[:, 128:256], 0: smath[:, 256:384]}

    # z: [128, 256 ch, 130 rho] fp16, rho innermost, zero halo cols.
    zbuf = persist.tile([128, 256, H + 2], F16)
    nc.vector.memset(zbuf[:, :, 0], 0.0)
    nc.vector.memset(zbuf[:, :, H + 1], 0.0)
    zview = zbuf.rearrange("p (r o) c -> p r o c", r=R)
    # gq: [128, 16 fields, 130] f32 (biases pre-added on evac; halo = 0)
    gq = persist.tile([128, 16, H + 2], F32)
    nc.vector.memset(gq[:, :, 0], 0.0)
    nc.vector.memset(gq[:, :, H + 1], 0.0)

    # normalized weights Wt[g, dxi, dyi, r, rho] fp16 + partition-shifted WQ
    Wt = persist.tile([128, 3, 3, R, H + 2], F16)
    WQp = persist.tile([128, 3, R, H + 2], F16)   # dx=+1 group shifted by -1
    WQm = persist.tile([128, 3, R, H + 2], F16)   # dx=-1 group shifted by +1
    WQ = {1: WQp, -1: WQm, 0: Wt[:, 1]}

    # ---------------- phase A ----------------
    with tc.tile_pool(name="phA", bufs=2) as pha, \
         tc.tile_pool(name="psA", bufs=2, space="PSUM") as psA, \
         tc.tile_pool(name="psG", bufs=2, space="PSUM") as psG:
        for k in range(H // SA):
            q0 = k * SA
            xw = pha.tile([128, SA * 128], BF16, tag="xw")
            nc.sync.dma_start(out=xw, in_=xslab_d[:, q0 * 128:(q0 + SA) * 128])
            xw2 = pha.tile([64, SA * 128], BF16, tag="xw2")
            nc.sync.dma_start(out=xw2, in_=x2_d[:, q0 * 128:(q0 + SA) * 128])
            gqp = psG.tile([128, SA * 16], F32, tag="gqp")
            for j2 in range(SA // 2):
                j = 2 * j2
                pt2 = psA.tile([128, 2, 256], F32, tag="pt2")
                for i in (0, 1):
                    lhsT = xw[:, (j + i) * 128:(j + i + 1) * 128]
                    nc.tensor.matmul(pt2[:, i], lhsT, wA_sb[:, 0:256],
                                     start=True, stop=True,
                                     skip_group_check=True)
                    g16 = gqp[:, (j + i) * 16:(j + i + 1) * 16]
                    # gq = (x0+x1+x2)^T (w0+w1+w2) to ~fp32 accuracy:
                    # [w0;w1]+[w1;w0]+[w2;w2] on [x0;x1], then x2^T w0.
                    nc.tensor.matmul(g16, lhsT, wA_sb[:, 256:272],
                                     start=True, stop=False,
                                     skip_group_check=True)
                    nc.tensor.matmul(g16, lhsT, wA_sb[:, 272:288],
                                     start=False, stop=False,
                                     skip_group_check=True)
                    nc.tensor.matmul(g16, lhsT, wA_sb[:, 288:304],
                                     start=False, stop=False,
                                     skip_group_check=True)
                    nc.tensor.matmul(g16, xw2[:, (j + i) * 128:(j + i + 1) * 128],
                                     wA_sb[0:64, 256:272],
                                     start=False, stop=True,
                                     skip_group_check=True)
                dst = zbuf[:, :, 1 + q0 + j:3 + q0 + j]   # [128, 256, 2]
                src = pt2.transpose([0, 2, 1])            # [128, 256, 2]
                if j2 % 2 == 0:
                    nc.scalar.activation(dst, src, AF.Copy)
                else:
                    nc.vector.tensor_copy(dst, src)
            # gq strip evac with bias add: psum [32, 16] -> gq [16, 32]
            dstg = gq[:, :, 1 + q0:1 + q0 + SA]
            srcg = gqp.rearrange("p (j c) -> p c j", c=16)
            bcol = gqbias[:, :, None].to_broadcast([128, 16, SA])
            nc.vector.tensor_tensor(dstg, srcg, bcol, op=OP.add)

    if dbg is not None:
        nc.sync.dma_start(out=dbg["dbg_gq"], in_=gq)
        nc.sync.dma_start(out=dbg["dbg_z"], in_=zbuf)

    # ---------------- fields (full image) ----------------
    with tc.tile_pool(name="phF", bufs=1) as fld, \
         tc.tile_pool(name="psF", bufs=2, space="PSUM") as psF, \
         tc.tile_pool(name="fsetup", bufs=1) as fst, \
         tc.tile_pool(name="ptmp", bufs=1) as ptp:

        def shift_into(dst_t, src_ap, sgn, dtype_f32, nch):
            """dst[g] = src[g+sgn]; src/dst [128, nch, 130]; 2-rule chunks."""
            step = 2
            for c0 in range(0, nch, step):
                ps = psF.tile([128, step, H + 2], F32, tag="psh")
                if dtype_f32:
                    nc.tensor.matmul(ps, SHIFT[sgn], src_ap[:, c0:c0 + step],
                                     start=True, stop=True,
                                     skip_group_check=True)
                else:
                    nc.tensor.matmul(ps, SHIFTH[sgn], src_ap[:, c0:c0 + step],
                                     start=True, stop=True,
                                     skip_group_check=True)
                if (c0 // step) % 2 == 0:
                    nc.scalar.activation(dst_t[:, c0:c0 + step], ps, AF.Copy)
                else:
                    nc.vector.tensor_copy(dst_t[:, c0:c0 + step], ps)

        thw = gq[:, 4:8, :]     # theta + b_th
        # --- theta path first (Sin table) ---
        m1 = fst.tile([128, R, H + 2], F32, tag="m1")
        m2 = fst.tile([128, R, H + 2], F32, tag="m2")
        tred = fst.tile([128, R, H + 2], F32, tag="tred")
        tred2 = fst.tile([128, R, H + 2], F32, tag="tred2")
        s2cF = fld.tile([128, R, H + 2], F32, tag="s2cF")
        c2cF = fld.tile([128, R, H + 2], F32, tag="c2cF")
        nc.vector.tensor_scalar(m1, thw, -PI / 2, None, op0=OP.is_lt)
        nc.vector.tensor_scalar(m2, thw, PI / 2, None, op0=OP.is_gt)
        nc.vector.tensor_tensor(m1, m1, m2, op=OP.subtract)
        nc.vector.scalar_tensor_tensor(out=tred, in0=m1, scalar=PI, in1=thw,
                                       op0=OP.mult, op1=OP.add)
        nc.scalar.activation(s2cF, tred, AF.Sin, scale=2.0)
        nc.vector.tensor_scalar(m1, thw, -0.75 * PI, None, op0=OP.is_lt)
        nc.vector.tensor_scalar(m2, thw, 0.25 * PI, None, op0=OP.is_gt)
        nc.vector.tensor_tensor(m1, m1, m2, op=OP.subtract)
        nc.vector.scalar_tensor_tensor(out=tred2, in0=m1, scalar=PI, in1=thw,
                                       op0=OP.mult, op1=OP.add)
        nc.scalar.activation(c2cF, tred2, AF.Sin, scale=2.0, bias=cb[:, 3:4])

        # --- softmax mu (Exp/Ln table) ---
        eg = fst.tile([128, R, H + 2], F32, tag="eg")
        nc.scalar.activation(eg, gq[:, 0:4, :], AF.Exp)
        nc.vector.memset(eg[:, :, 0], 0.0)
        nc.vector.memset(eg[:, :, H + 1], 0.0)
        zsum = fst.tile([128, H + 2], F32, tag="zsum")
        nc.vector.tensor_tensor(zsum, eg[:, 0], eg[:, 1], op=OP.add)
        nc.vector.tensor_tensor(zsum, zsum, eg[:, 2], op=OP.add)
        nc.vector.tensor_tensor(zsum, zsum, eg[:, 3], op=OP.add)
        rz = fst.tile([128, H + 2], F32, tag="rz")
        nc.scalar.activation(rz, zsum, AF.Ln, bias=cb[:, 0:1])
        nc.scalar.activation(rz, rz, AF.Exp, scale=-1.0)
        mu = fld.tile([128, R, H + 2], F32, tag="mu")
        rzb = rz[:, None, :].to_broadcast([128, R, H + 2])
        nc.vector.tensor_tensor(mu, eg, rzb, op=OP.mult)

        # --- hyper / base fields ---
        uh = fst.tile([128, R, H + 2], F32, tag="uh")
        nc.scalar.activation(uh, gq[:, 12:16, :], AF.Exp)
        ub = fst.tile([128, R, H + 2], F32, tag="ub")
        nc.scalar.activation(ub, gq[:, 8:12, :], AF.Exp)
        Ft = fld.tile([128, R, H + 2], F32, tag="Ft")
        nc.vector.tensor_scalar_add(Ft, uh, 1.0)
        lnf = fst.tile([128, R, H + 2], F32, tag="lnf")
        nc.scalar.activation(lnf, uh, AF.Ln, bias=1.0)
        Gt = fld.tile([128, R, H + 2], F32, tag="Gt")
        nc.scalar.activation(Gt, lnf, AF.Exp, scale=-1.0)
        bt = fld.tile([128, R, H + 2], F32, tag="bt")
        nc.scalar.activation(bt, ub, AF.Ln, bias=1.0)

        if dbg is not None:
            nc.sync.dma_start(out=dbg["dbg_mu"], in_=mu)

        # --- shifted copies ---
        base = {"c2c": c2cF, "s2c": s2cF, "Ft": Ft, "Gt": Gt, "bt": bt}
        shifted = {}
        for name, t in base.items():
            d = {0: t}
            for sgn in (1, -1):
                st = fld.tile([128, R, H + 2], F32, tag=f"{name}s{sgn}")
                shift_into(st, t, sgn, True, R)
                d[sgn] = st
            shifted[name] = d
        mu16 = fld.tile([128, R, H + 2], F16, tag="mu16")
        nc.vector.tensor_copy(mu16, mu)
        mup = fld.tile([128, R, H + 2], F16, tag="mup")
        mum = fld.tile([128, R, H + 2], F16, tag="mum")
        shift_into(mup, mu16, 1, False, R)
        shift_into(mum, mu16, -1, False, R)
        MUSH = {0: mu16, 1: mup, -1: mum}

        # --- pair loop ---
        comu = [fld.tile([128, R, H + 2], F16, tag=f"comu{i}",
                         name=f"comu{i}") for i in range(4)]
        for cm in comu:
            nc.vector.memset(cm[:, :, 0], 0.0)
            nc.vector.memset(cm[:, :, H + 1], 0.0)
        den = fld.tile([128, R, H], F32, tag="den")
        compat_t = {}
        mirror_t = {}

        def Ctr(t):
            return t[:, :, 1:1 + H]

        for ip, (dy, dx) in enumerate(PAIRS):
            def S(name):
                return shifted[name][dx][:, :, 1 + dy:1 + dy + H]

            c2 = ptp.tile([128, R, H], F32, tag="c2")
            s2 = ptp.tile([128, R, H], F32, tag="s2")
            q = ptp.tile([128, R, H], F32, tag="q")
            t1 = ptp.tile([128, R, H], F32, tag="t1")
            nc.vector.tensor_tensor(c2, Ctr(c2cF), S("c2c"), op=OP.add)
            nc.vector.tensor_tensor(s2, Ctr(s2cF), S("s2c"), op=OP.add)
            nc.vector.tensor_tensor(q, c2, c2, op=OP.mult)
            nc.vector.tensor_tensor(t1, s2, s2, op=OP.mult)
            nc.vector.tensor_tensor(q, q, t1, op=OP.add)
            rin = ptp.tile([128, R, H], F32, tag="rin")
            nc.scalar.activation(rin, q, AF.Ln)
            nc.scalar.activation(rin, rin, AF.Exp, scale=-0.5)
            nc.vector.tensor_scalar(rin, rin, 1e6, None, op0=OP.min)
            nc.vector.tensor_tensor(c2, c2, rin, op=OP.mult)
            nc.vector.tensor_tensor(s2, s2, rin, op=OP.mult)
            E = ptp.tile([128, R, H], F32, tag="E")
            iE = ptp.tile([128, R, H], F32, tag="iE")
            bp = ptp.tile([128, R, H], F32, tag="bp")
            nc.vector.tensor_tensor(E, Ctr(Ft), S("Ft"), op=OP.mult)
            nc.vector.tensor_tensor(iE, Ctr(Gt), S("Gt"), op=OP.mult)
            nc.vector.tensor_tensor(bp, Ctr(bt), S("bt"), op=OP.add)
            rbp = ptp.tile([128, R, H], F32, tag="rbp")
            nc.scalar.activation(rbp, bp, AF.Ln, bias=cb[:, 1:2])
            nc.scalar.activation(rbp, rbp, AF.Exp, scale=-2.0)
            pu2 = ptp.tile([128, R, H], F32, tag="pu2")
            ps2 = ptp.tile([128, R, H], F32, tag="ps2")
            a1, a2, a3 = dx * dx, dy * dy, dx * dy
            if a3 == 0:
                hc = 0.5 * (a1 - a2)
                nc.vector.tensor_scalar(pu2, c2, hc, 0.5, op0=OP.mult,
                                        op1=OP.add)
                nc.vector.tensor_scalar(ps2, c2, -hc, 0.5, op0=OP.mult,
                                        op1=OP.add)
            else:
                nc.vector.tensor_scalar(pu2, s2, float(a3), 1.0, op0=OP.mult,
                                        op1=OP.add)
                nc.vector.tensor_scalar(ps2, s2, float(-a3), 1.0, op0=OP.mult,
                                        op1=OP.add)
            nc.vector.tensor_tensor(pu2, pu2, iE, op=OP.mult)
            nc.vector.tensor_tensor(ps2, ps2, E, op=OP.mult)
            nc.vector.tensor_tensor(pu2, pu2, ps2, op=OP.add)
            nc.vector.tensor_tensor(pu2, pu2, rbp, op=OP.mult)
            kern = ptp.tile([128, R, H], F32, tag="kern")
            nc.scalar.activation(kern, pu2, AF.Exp, scale=-4.0)

            nc.vector.tensor_tensor(comu[ip][:, :, 1:1 + H], kern, Ctr(mu),
                                    op=OP.mult)
            cp = fld.tile([128, R, H], F32, tag=f"cp{ip}")
            nc.vector.tensor_tensor(
                cp, kern, MUSH[dx][:, :, 1 + dy:1 + dy + H], op=OP.mult)
            compat_t[ip] = cp
            if ip == 0:
                nc.vector.tensor_tensor(den, Ctr(mu), cp, op=OP.add)
            else:
                nc.vector.tensor_tensor(den, den, cp, op=OP.add)
            # mirror compat = comu shifted by (-dy, -dx)
            if dx != 0:
                cst = fld.tile([128, R, H + 2], F16, tag=f"csh{ip}")
                shift_into(cst, comu[ip], -dx, False, R)
                mirror = cst[:, :, 1 - dy:1 - dy + H]
            else:
                mirror = comu[ip][:, :, 1 - dy:1 - dy + H]
            mirror_t[ip] = mirror
            nc.vector.tensor_tensor(den, den, mirror, op=OP.add)

        if dbg is not None:
            nc.sync.dma_start(out=dbg["dbg_den"], in_=den)
        rden = fld.tile([128, R, H], F32, tag="rden")
        nc.scalar.activation(rden, den, AF.Ln, bias=cb[:, 2:3])
        nc.scalar.activation(rden, rden, AF.Exp, scale=-1.0)

        # --- normalized weights into Wt ---
        for ip, (dy, dx) in enumerate(PAIRS):
            nc.vector.tensor_tensor(Wt[:, DXI[dx], 1 + dy, :, 1:1 + H],
                                    compat_t[ip], rden, op=OP.mult)
            nc.vector.tensor_tensor(Wt[:, DXI[-dx], 1 - dy, :, 1:1 + H],
                                    mirror_t[ip], rden, op=OP.mult)
        nc.vector.tensor_tensor(Wt[:, 1, 1, :, 1:1 + H], Ctr(mu), rden,
                                op=OP.mult)
        if dbg is not None:
            nc.sync.dma_start(out=dbg["dbg_wt"], in_=Wt)

        # --- partition-shift dx groups: WQ[dx][g] = Wt[dx-group][g-dx] ---
        for dx, wq in ((1, WQp), (-1, WQm)):
            src = Wt[:, DXI[dx]].rearrange("p a r c -> p (a r) c")
            dst = wq.rearrange("p a r c -> p (a r) c")
            shift_into(dst, src, -dx, False, 3 * R)

    # ---------------- stencil ----------------
    with tc.tile_pool(name="phC", bufs=2) as pc, \
         tc.tile_pool(name="psC", bufs=2, space="PSUM") as psC:
        for b in range(NBLK):
            r0 = b * BLK
            P = {}
            for dx in (-1, 0, 1):
                Pt = pc.tile([128, 3, R, Cout, BLK], F16, tag=f"P{dx}")
                for dyi, dy in enumerate((-1, 0, 1)):
                    wb = WQ[dx][:, dyi, :, None,
                                1 + r0:1 + r0 + BLK].to_broadcast(
                        [128, R, Cout, BLK])
                    nc.vector.tensor_tensor(
                        Pt[:, dyi],
                        zview[:, :, :, r0 + dy + 1:r0 + dy + 1 + BLK],
                        wb, op=OP.mult)
                P[dx] = Pt

            acc = psC.tile([128, BLK, Cout], F32, tag="acc")
            accT = acc.transpose([0, 2, 1])   # [128, 64, 8] view
            n = 0
            for dx in (-1, 0, 1):
                for dyi in range(3):
                    for r in range(R):
                        rhs = P[dx][:, dyi, r]       # [128, 64, 8] contiguous
                        nc.tensor.matmul(accT, SHIFTH[dx], rhs,
                                         start=(n == 0), stop=False,
                                         skip_group_check=True)
                        n += 1
            nc.tensor.matmul(acc, aux[:, 0:128], aux[:, 128:640],
                             start=False, stop=True, skip_group_check=True)
            stg = pc.tile([128, BLK, Cout], F32, tag="stg")
            nc.scalar.activation(stg, acc, AF.Copy)
            dst = out_d[r0 * 128:(r0 + BLK) * 128, :].rearrange(
                "(rho g) o -> g rho o", g=128)
            nc.sync.dma_start(out=dst, in_=stg)


def _host_prep(inputs):
    import ml_dtypes
    bf = ml_dtypes.bfloat16
    x = np.asarray(inputs["x"], np.float32)
    gate_w = np.asarray(inputs["gate_w"], np.float32)
    gate_b = np.asarray(inputs["gate_b"], np.float32)
    value_w = np.asarray(inputs["value_w"], np.float32)
    geom_w = np.asarray(inputs["geom_w"], np.float32)
    geom_b = np.asarray(inputs["geom_b"], np.float32)
    pw_w = np.asarray(inputs["pw_w"], np.float32)
    pw_b = np.asarray(inputs["pw_b"], np.float32)

    M = pw_w.reshape(Cout, R, C).transpose(1, 0, 2) @ value_w      # [R,Cout,C]
    wz = M.transpose(2, 0, 1).reshape(C, R * Cout)                 # [C, 256]
    wgq = np.concatenate([gate_w.T, geom_w.T], axis=1)             # [C, 16]

    xf = np.ascontiguousarray(x.reshape(B, C, L))
    xh = xf.astype(bf)
    xl = (xf - xh.astype(np.float32)).astype(bf)
    x2 = (xf - xh.astype(np.float32) - xl.astype(np.float32)).astype(bf)
    xslab = np.concatenate([xh, xl], axis=1)                       # [B,128,L]

    w2h = wgq.astype(bf)
    w2l = (wgq - w2h.astype(np.float32)).astype(bf)
    w2q = (wgq - w2h.astype(np.float32) - w2l.astype(np.float32)).astype(bf)
    wA = np.zeros((128, 304), np.float32)
    wA[0:64, 0:256] = wz
    wA[0:64, 256:272] = w2h.astype(np.float32)
    wA[64:128, 256:272] = w2l.astype(np.float32)
    wA[0:64, 272:288] = w2l.astype(np.float32)
    wA[64:128, 272:288] = w2h.astype(np.float32)
    wA[0:64, 288:304] = w2q.astype(np.float32)
    wA[64:128, 288:304] = w2q.astype(np.float32)

    smat = np.zeros((128, 384), np.float32)
    for g in range(128):   # Sp[k, g] = 1 iff k = g+1 ; Sm[k, g] = 1 iff k=g-1
        if g + 1 < 128:
            smat[g + 1, g] = 1.0
        if g - 1 >= 0:
            smat[g - 1, 128 + g] = 1.0
        smat[g, 256 + g] = 1.0

    aux = np.zeros((1, 640), np.float32)
    aux[0, 0:128] = 1.0
    aux[0, 128:640] = np.tile(pw_b[None, :], (BLK, 1)).reshape(-1)

    gqb_cols = np.concatenate([gate_b, geom_b])                    # [16]
    gqbias = np.tile(gqb_cols[None, :], (128, 1)).astype(np.float32)

    return {
        "xslab": xslab,
        "x2slab": x2,
        "wA": wA.astype(bf),
        "smat": smat[:, 0:256].copy(),
        "smath": smat.astype(np.float16),
        "gqbias": gqbias,
        "aux": aux.astype(bf),
    }


def make_in_maps(inputs):
    h = _host_prep(inputs)
    return [{"xslab": h["xslab"][b], "x2slab": h["x2slab"][b], "wA": h["wA"],
             "smat": h["smat"], "smath": h["smath"], "gqbias": h["gqbias"],
             "aux": h["aux"]} for b in range(B)]


def kernel(**inputs) -> np.ndarray:
    if "nc" not in _CACHE:
        _CACHE["nc"] = build_program()
    nc = _CACHE["nc"]
    in_maps = make_in_maps(inputs)
    res = run_bass_kernel_spmd(nc, in_maps, core_ids=list(range(NCORE)))
    out = np.stack([
        res.results[b]["out"].reshape(H, W, Cout).transpose(2, 0, 1)
        for b in range(B)
    ])
    return out.astype(np.float32)


# revision 34
# speedup vs baseline: 2.1674x; 1.1546x over previous
"""Trainium2 Bass kernel for nn_AZConv2d (fuzzy-rule hyperbolic-geometry message passing).

Self-contained: hardcodes shapes B=8,C=64,H=W=128,R=4,Cout=64; shards batch over 8 cores.

v7 (861us -> 400us vs v2): phase-split pipeline, all-bf16/fp16 matmul streams.
  - Pass A1 (gq only): per row one bf16 stationary [x_hi; x_lo] + 4 small
    matmuls giving gq = (x0+x1+x2)^T(w0+w1+w2) to ~fp32 accuracy (needed:
    theta pairs can be degenerate to ~1e-5); biases folded into the strip
    -granular PSUM->SBUF evac add. No fp32 LDWEIGHTS anywhere.
  - Fields: full-image [128, 4, 130] ops; Sin ops grouped before Exp/Ln ops
    (ACT table swaps); pair-loop temps double-buffered so pairs pipeline.
  - Pass A2 (z): per ROW-PAIR one N=512 matmul ([x0_j; x0_j+1] stationary vs
    block-diag [[wz,0],[0,wz]]); evacs ACT-only so the DVE stays free; z in
    4 strip tiles [128, 256ch, 34rho] fp16 (seam rows duplicated) so stencil
    products depend per-strip, not on the whole-image z; pass A2's pools stay
    open through the stencil (SBUF/PSUM space reuse would serialize phases).
    Its matmuls fill the PE while the DVE does fields; its evacs hide under
    the DVE-bound stencil.
  - Stencil: products on DVE (fp16 2x, ~16us/block is the wall) into
    P[dx][128, 3dy, 4r, 64o, 8rho], rho innermost everywhere; 36-term
    (dx,dy,r) sum as PSUM-accumulating shift matmuls with CONTIGUOUS rhs
    (N=512) and CONTIGUOUS dst acc[128, Cout, BLK] (a transposed matmul dst
    halves the PE rate: 429 vs 216ns measured); ACT evac un-transposes.
  - dedupe_ldweights(): the scheduler emits one LDWEIGHTS per matmul; repeats
    of the identical stationary are rewritten to NoOps post-schedule.
"""
import numpy as np
from contextlib import ExitStack

import concourse.bass as bass
import concourse.tile as tile
from concourse import mybir
from concourse.bass_utils import run_bass_kernel_spmd

F32 = mybir.dt.float32
F16 = mybir.dt.float16
BF16 = mybir.dt.bfloat16
AF = mybir.ActivationFunctionType
OP = mybir.AluOpType

B, C, H, W, R, Cout = 8, 64, 128, 128, 4, 64
L = H * W
NCORE = 8
BLK = 8                 # stencil rows per psum accumulation block
NBLK = H // BLK         # 16
SA = 32                 # phase A strip rows
PI = float(np.pi)
PAIRS = [(0, 1), (1, -1), (1, 0), (1, 1)]   # (dy, dx)
DXI = {-1: 0, 0: 1, 1: 2}

_CACHE = {}


def split_multiwaits(nc):
    """This walrus accepts ONE sync wait per instruction: split extras into
    same-engine NoOps inserted just before the instruction."""
    n = 0
    for bb in nc.main_func.blocks:
        out = []
        for ins in bb.instructions:
            si = ins.sync_info
            if si is not None and len(si.on_wait) > 1:
                waits = list(si.on_wait)
                for w in waits[:-1]:
                    n += 1
                    nop = mybir.InstNoOp(name=f"WSPLIT-{n}")
                    nop.engine = ins.engine
                    nop.sync_info = mybir.SyncInfo(on_wait=[w], on_update=[])
                    out.append(nop)
                ins.sync_info = mybir.SyncInfo(on_wait=[waits[-1]],
                                               on_update=list(si.on_update))
            out.append(ins)
        bb.instructions[:] = out
    return n


def dedupe_ldweights(nc):
    """The tile scheduler emits one LDWEIGHTS per matmul even when many
    consecutive matmuls stream against the identical stationary (e.g. the 12
    shift matmuls per dx group). A reload of the already-loaded array costs
    ~215ns and serializes with the stream. Convert LDWEIGHTS whose weight AP
    (and tile cfg) matches the previous PE weight load into NoOps, keeping
    sync_info so semaphore semantics are unchanged."""
    n = 0
    for bb in nc.main_func.blocks:
        last_sig = None
        out = []
        for ins in bb.instructions:
            tn = type(ins).__name__
            if tn == 'InstLdweights':
                sig = (str(ins.ins[0]),
                       str(getattr(ins, 'tile_position', None)),
                       str(getattr(ins, 'tile_size', None)),
                       str(getattr(ins, 'perf_mode', None)),
                       str(getattr(ins, 'is_transpose', None)))
                if sig == last_sig:
                    n += 1
                    nop = mybir.InstNoOp(name=f"LWDEDUP-{n}")
                    nop.engine = ins.engine
                    nop.sync_info = ins.sync_info
                    out.append(nop)
                    continue
                last_sig = sig
            out.append(ins)
        bb.instructions[:] = out
    return n


def build_program(debug=False):
    nc = bass.Bass()
    xslab_d = nc.dram_tensor("xslab", [128, L], BF16, kind="ExternalInput")
    x2_d = nc.dram_tensor("x2slab", [64, L], BF16, kind="ExternalInput")
    xz_d = nc.dram_tensor("xzslab", [128, L // 2], BF16, kind="ExternalInput")
    wz2_d = nc.dram_tensor("wz2", [128, 512], BF16, kind="ExternalInput")
    wA_d = nc.dram_tensor("wA", [128, 304], BF16, kind="ExternalInput")
    smat_d = nc.dram_tensor("smat", [128, 256], F32, kind="ExternalInput")
    smath_d = nc.dram_tensor("smath", [128, 384], F16, kind="ExternalInput")
    gqb_d = nc.dram_tensor("gqbias", [128, 16], F32, kind="ExternalInput")
    aux_d = nc.dram_tensor("aux", [1, 640], BF16, kind="ExternalInput")
    out_d = nc.dram_tensor("out", [L, Cout], F32, kind="ExternalOutput")
    dbg = None
    if debug:
        dbg = {
            "dbg_gq": nc.dram_tensor("dbg_gq", [128, 16, H + 2], F32,
                                     kind="ExternalOutput")[:],
            "dbg_z": nc.dram_tensor("dbg_z", [128, 256, H + 2], F16,
                                    kind="ExternalOutput")[:],
            "dbg_mu": nc.dram_tensor("dbg_mu", [128, R, H + 2], F32,
                                     kind="ExternalOutput")[:],
            "dbg_wt": nc.dram_tensor("dbg_wt", [128, 3, 3, R, H + 2], F16,
                                     kind="ExternalOutput")[:],
            "dbg_den": nc.dram_tensor("dbg_den", [128, R, H], F32,
                                      kind="ExternalOutput")[:],
        }

    with ExitStack() as ctx:
        tc = ctx.enter_context(tile.TileContext(nc))
        _emit(ctx, tc, xslab_d[:], x2_d[:], xz_d[:], wz2_d[:], wA_d[:],
              smat_d[:], smath_d[:], gqb_d[:], aux_d[:], out_d[:], dbg)
    ndup = dedupe_ldweights(nc)
    split_multiwaits(nc)
    if ndup == 0:
        log_msg = "dedupe_ldweights removed nothing"
    return nc


def _emit(ctx, tc, xslab_d, x2_d, xz_d, wz2_d, wA_d, smat_d, smath_d, gqb_d,
          aux_d, out_d, dbg=None):
    nc = tc.nc

    persist = ctx.enter_context(tc.tile_pool(name="persist", bufs=1))

    # ---------------- persistent tensors ----------------
    wA_sb = persist.tile([128, 304], BF16)
    nc.sync.dma_start(out=wA_sb, in_=wA_d)
    wz2_sb = persist.tile([128, 512], BF16)
    nc.sync.dma_start(out=wz2_sb, in_=wz2_d)
    smat = persist.tile([128, 256], F32)       # [Sp | Sm] f32
    nc.sync.dma_start(out=smat, in_=smat_d)
    smath = persist.tile([128, 384], F16)      # [Sp | Sm | I] fp16
    nc.sync.dma_start(out=smath, in_=smath_d)
    gqbias = persist.tile([128, 16], F32)
    nc.sync.dma_start(out=gqbias, in_=gqb_d)
    aux = persist.tile([1, 640], BF16)         # [ones(128) | pwb_row(512)]
    nc.sync.dma_start(out=aux, in_=aux_d)

    # bias constants for ACT ops ([P,1] APs)
    cb = persist.tile([128, 4], F32)
    nc.vector.memset(cb[:, 0:1], 1e-30)
    nc.vector.memset(cb[:, 1:2], 2e-4)
    nc.vector.memset(cb[:, 2:3], 1e-6)
    nc.vector.memset(cb[:, 3:4], float(np.pi / 2))

    SHIFT = {1: smat[:, 0:128], -1: smat[:, 128:256]}
    SHIFTH = {1: smath[:, 0:128], -1: smath[:, 128:256], 0: smath[:, 256:384]}

    # z in 4 strip tiles [128, 256 ch, 34 rho] fp16 (rho innermost; col c of
    # tile k = image row 32k-1+c, one halo row duplicated at each seam).
    # Strip granularity lets stencil products start as soon as their strip's
    # rows are evacuated instead of waiting for the whole-image z.
    zs = [persist.tile([128, 256, SA + 2], F16, name=f"zs{k}")
          for k in range(H // SA)]
    nc.vector.memset(zs[0][:, :, 0], 0.0)
    nc.vector.memset(zs[H // SA - 1][:, :, SA + 1], 0.0)
    zsv = [t.rearrange("p (r o) c -> p r o c", r=R) for t in zs]
    # gq: [128, 16 fields, 130] f32 (biases pre-added on evac; halo = 0)
    gq = persist.tile([128, 16, H + 2], F32)
    nc.vector.memset(gq[:, :, 0], 0.0)
    nc.vector.memset(gq[:, :, H + 1], 0.0)

    # normalized weights Wt[g, dxi, dyi, r, rho] fp16 + partition-shifted WQ
    Wt = persist.tile([128, 3, 3, R, H + 2], F16)
    WQp = persist.tile([128, 3, R, H + 2], F16)   # dx=+1 group shifted by -1
    WQm = persist.tile([128, 3, R, H + 2], F16)   # dx=-1 group shifted by +1
    WQ = {1: WQp, -1: WQm, 0: Wt[:, 1]}

    # ---------------- phase A1: gq only ----------------
    # (z is a separate pass emitted after the field ops so its matmuls fill
    # the PE while the DVE chews on fields; its evacs go ACT-only and overlap
    # the stencil.)
    with tc.tile_pool(name="phG", bufs=2) as phg, \
         tc.tile_pool(name="psG", bufs=2, space="PSUM") as psG:
        for k in range(H // SA):
            q0 = k * SA
            xw = phg.tile([128, SA * 128], BF16, tag="xw")
            nc.sync.dma_start(out=xw, in_=xslab_d[:, q0 * 128:(q0 + SA) * 128])
            xw2 = phg.tile([64, SA * 128], BF16, tag="xw2")
            nc.sync.dma_start(out=xw2, in_=x2_d[:, q0 * 128:(q0 + SA) * 128])
            gqp = psG.tile([128, SA * 16], F32, tag="gqp")
            for j in range(SA):
                lhsT = xw[:, j * 128:(j + 1) * 128]
                g16 = gqp[:, j * 16:(j + 1) * 16]
                # gq = (x0+x1+x2)^T (w0+w1+w2) to ~fp32 accuracy:
                # [w0;w1]+[w1;w0]+[w2;w2] on [x0;x1], then x2^T w0.
                nc.tensor.matmul(g16, lhsT, wA_sb[:, 256:272],
                                 start=True, stop=False,
                                 skip_group_check=True)
                nc.tensor.matmul(g16, lhsT, wA_sb[:, 272:288],
                                 start=False, stop=False,
                                 skip_group_check=True)
                nc.tensor.matmul(g16, lhsT, wA_sb[:, 288:304],
                                 start=False, stop=False,
                                 skip_group_check=True)
                nc.tensor.matmul(g16, xw2[:, j * 128:(j + 1) * 128],
                                 wA_sb[0:64, 256:272],
                                 start=False, stop=True,
                                 skip_group_check=True)
            # gq strip evac with bias add: psum [32, 16] -> gq [16, 32]
            dstg = gq[:, :, 1 + q0:1 + q0 + SA]
            srcg = gqp.rearrange("p (j c) -> p c j", c=16)
            bcol = gqbias[:, :, None].to_broadcast([128, 16, SA])
            nc.vector.tensor_tensor(dstg, srcg, bcol, op=OP.add)

    if dbg is not None:
        nc.sync.dma_start(out=dbg["dbg_gq"], in_=gq)

    # ---------------- phase A2 setup: z pass (one N=512 matmul per row pair:
    # [x0_j; x0_j+1] stationary vs block-diag [[wz,0],[0,wz]]; ACT-only evacs
    # into per-strip z tiles). These pools stay open through the stencil so
    # the stencil's P-tile pool does NOT reuse their SBUF/PSUM space - space
    # reuse would make the first product wait for the LAST z matmul. ------
    phz = ctx.enter_context(tc.tile_pool(name="phZ", bufs=2))
    psZ = ctx.enter_context(tc.tile_pool(name="psZ", bufs=4, space="PSUM"))

    def emit_z_strip(k):
        q0 = k * SA
        xzw = phz.tile([128, (SA // 2) * 128], BF16, tag="xzw", name="xzw")
        nc.sync.dma_start(
            out=xzw, in_=xz_d[:, (q0 // 2) * 128:(q0 // 2 + SA // 2) * 128])
        for p in range(SA // 2):
            pt2 = psZ.tile([128, 2, 256], F32, tag="pt2", name="pt2")
            nc.tensor.matmul(pt2, xzw[:, p * 128:(p + 1) * 128], wz2_sb,
                             start=True, stop=True, skip_group_check=True)
            srcT = pt2.transpose([0, 2, 1])               # [128, 256, 2]
            dst = zs[k][:, :, 1 + 2 * p:3 + 2 * p]
            nc.scalar.activation(dst, srcT, AF.Copy)
            if p == 0 and k > 0:              # row 32k = prev tile's col 33
                nc.scalar.activation(zs[k - 1][:, :, SA + 1:SA + 2],
                                     srcT[:, :, 0:1], AF.Copy)
            if p == SA // 2 - 1 and k < H // SA - 1:
                # row 32k+31 = next tile's col 0
                nc.scalar.activation(zs[k + 1][:, :, 0:1],
                                     srcT[:, :, 1:2], AF.Copy)

    emit_z_strip(0)

    # ---------------- fields (full image) ----------------
    with tc.tile_pool(name="phF", bufs=1) as fld, \
         tc.tile_pool(name="psF", bufs=4, space="PSUM") as psF:
        fst = ctx.enter_context(tc.tile_pool(name="fsetup", bufs=1))

        def shift_into(dst_t, src_ap, sgn, dtype_f32, nch):
            """dst[g] = src[g+sgn]; src/dst [128, nch, 130]; 2-rule chunks."""
            step = 2
            for c0 in range(0, nch, step):
                ps = psF.tile([128, step, H + 2], F32, tag="psh")
                if dtype_f32:
                    nc.tensor.matmul(ps, SHIFT[sgn], src_ap[:, c0:c0 + step],
                                     start=True, stop=True,
                                     skip_group_check=True)
                else:
                    nc.tensor.matmul(ps, SHIFTH[sgn], src_ap[:, c0:c0 + step],
                                     start=True, stop=True,
                                     skip_group_check=True)
                nc.scalar.activation(dst_t[:, c0:c0 + step], ps, AF.Copy)

        thw = gq[:, 4:8, :]     # theta + b_th
        # --- theta path first (Sin table) ---
        m1 = fst.tile([128, R, H + 2], F32, tag="m1")
        m2 = fst.tile([128, R, H + 2], F32, tag="m2")
        tred = fst.tile([128, R, H + 2], F32, tag="tred")
        tred2 = fst.tile([128, R, H + 2], F32, tag="tred2")
        s2cF = fld.tile([128, R, H + 2], F32, tag="s2cF")
        c2cF = fld.tile([128, R, H + 2], F32, tag="c2cF")
        nc.vector.tensor_scalar(m1, thw, -PI / 2, None, op0=OP.is_lt)
        nc.vector.tensor_scalar(m2, thw, PI / 2, None, op0=OP.is_gt)
        nc.vector.tensor_tensor(m1, m1, m2, op=OP.subtract)
        nc.vector.scalar_tensor_tensor(out=tred, in0=m1, scalar=PI, in1=thw,
                                       op0=OP.mult, op1=OP.add)
        nc.scalar.activation(s2cF, tred, AF.Sin, scale=2.0)
        nc.vector.tensor_scalar(m1, thw, -0.75 * PI, None, op0=OP.is_lt)
        nc.vector.tensor_scalar(m2, thw, 0.25 * PI, None, op0=OP.is_gt)
        nc.vector.tensor_tensor(m1, m1, m2, op=OP.subtract)
        nc.vector.scalar_tensor_tensor(out=tred2, in0=m1, scalar=PI, in1=thw,
                                       op0=OP.mult, op1=OP.add)
        nc.scalar.activation(c2cF, tred2, AF.Sin, scale=2.0, bias=cb[:, 3:4])

        # --- softmax mu (Exp/Ln table) ---
        eg = fst.tile([128, R, H + 2], F32, tag="eg")
        nc.scalar.activation(eg, gq[:, 0:4, :], AF.Exp)
        nc.vector.memset(eg[:, :, 0], 0.0)
        nc.vector.memset(eg[:, :, H + 1], 0.0)
        zsum = fst.tile([128, H + 2], F32, tag="zsum")
        nc.vector.tensor_tensor(zsum, eg[:, 0], eg[:, 1], op=OP.add)
        nc.vector.tensor_tensor(zsum, zsum, eg[:, 2], op=OP.add)
        nc.vector.tensor_tensor(zsum, zsum, eg[:, 3], op=OP.add)
        rz = fst.tile([128, H + 2], F32, tag="rz")
        nc.scalar.activation(rz, zsum, AF.Ln, bias=cb[:, 0:1])
        nc.scalar.activation(rz, rz, AF.Exp, scale=-1.0)
        mu = fld.tile([128, R, H + 2], F32, tag="mu")
        rzb = rz[:, None, :].to_broadcast([128, R, H + 2])
        nc.vector.tensor_tensor(mu, eg, rzb, op=OP.mult)

        # --- hyper / base fields ---
        uh = fst.tile([128, R, H + 2], F32, tag="uh")
        nc.scalar.activation(uh, gq[:, 12:16, :], AF.Exp)
        ub = fst.tile([128, R, H + 2], F32, tag="ub")
        nc.scalar.activation(ub, gq[:, 8:12, :], AF.Exp)
        Ft = fld.tile([128, R, H + 2], F32, tag="Ft")
        nc.vector.tensor_scalar_add(Ft, uh, 1.0)
        lnf = fst.tile([128, R, H + 2], F32, tag="lnf")
        nc.scalar.activation(lnf, uh, AF.Ln, bias=1.0)
        Gt = fld.tile([128, R, H + 2], F32, tag="Gt")
        nc.scalar.activation(Gt, lnf, AF.Exp, scale=-1.0)
        bt = fld.tile([128, R, H + 2], F32, tag="bt")
        nc.scalar.activation(bt, ub, AF.Ln, bias=1.0)

        if dbg is not None:
            nc.sync.dma_start(out=dbg["dbg_mu"], in_=mu)

        # --- shifted copies ---
        base = {"c2c": c2cF, "s2c": s2cF, "Ft": Ft, "Gt": Gt, "bt": bt}
        shifted = {}
        for name, t in base.items():
            d = {0: t}
            for sgn in (1, -1):
                st = fld.tile([128, R, H + 2], F32, tag=f"{name}s{sgn}")
                shift_into(st, t, sgn, True, R)
                d[sgn] = st
            shifted[name] = d
        mu16 = fld.tile([128, R, H + 2], F16, tag="mu16")
        nc.vector.tensor_copy(mu16, mu)
        mup = fld.tile([128, R, H + 2], F16, tag="mup")
        mum = fld.tile([128, R, H + 2], F16, tag="mum")
        shift_into(mup, mu16, 1, False, R)
        shift_into(mum, mu16, -1, False, R)
        MUSH = {0: mu16, 1: mup, -1: mum}

        ptp = ctx.enter_context(tc.tile_pool(name="ptmp", bufs=2))

        # --- pair loop ---
        comu = [fld.tile([128, R, H + 2], F16, tag=f"comu{i}",
                         name=f"comu{i}") for i in range(4)]
        for cm in comu:
            nc.vector.memset(cm[:, :, 0], 0.0)
            nc.vector.memset(cm[:, :, H + 1], 0.0)
        den = fld.tile([128, R, H], F32, tag="den")
        compat_t = {}
        mirror_t = {}

        def Ctr(t):
            return t[:, :, 1:1 + H]

        for ip, (dy, dx) in enumerate(PAIRS):
            def S(name):
                return shifted[name][dx][:, :, 1 + dy:1 + dy + H]

            c2 = ptp.tile([128, R, H], F32, tag="c2")
            s2 = ptp.tile([128, R, H], F32, tag="s2")
            q = ptp.tile([128, R, H], F32, tag="q")
            t1 = ptp.tile([128, R, H], F32, tag="t1")
            nc.vector.tensor_tensor(c2, Ctr(c2cF), S("c2c"), op=OP.add)
            nc.vector.tensor_tensor(s2, Ctr(s2cF), S("s2c"), op=OP.add)
            nc.vector.tensor_tensor(q, c2, c2, op=OP.mult)
            nc.vector.tensor_tensor(t1, s2, s2, op=OP.mult)
            nc.vector.tensor_tensor(q, q, t1, op=OP.add)
            rin = ptp.tile([128, R, H], F32, tag="rin")
            nc.scalar.activation(rin, q, AF.Ln)
            nc.scalar.activation(rin, rin, AF.Exp, scale=-0.5)
            nc.vector.tensor_scalar(rin, rin, 1e6, None, op0=OP.min)
            nc.vector.tensor_tensor(c2, c2, rin, op=OP.mult)
            nc.vector.tensor_tensor(s2, s2, rin, op=OP.mult)
            E = ptp.tile([128, R, H], F32, tag="E")
            iE = ptp.tile([128, R, H], F32, tag="iE")
            bp = ptp.tile([128, R, H], F32, tag="bp")
            nc.vector.tensor_tensor(E, Ctr(Ft), S("Ft"), op=OP.mult)
            nc.vector.tensor_tensor(iE, Ctr(Gt), S("Gt"), op=OP.mult)
            nc.vector.tensor_tensor(bp, Ctr(bt), S("bt"), op=OP.add)
            rbp = ptp.tile([128, R, H], F32, tag="rbp")
            nc.scalar.activation(rbp, bp, AF.Ln, bias=cb[:, 1:2])
            nc.scalar.activation(rbp, rbp, AF.Exp, scale=-2.0)
            pu2 = ptp.tile([128, R, H], F32, tag="pu2")
            ps2 = ptp.tile([128, R, H], F32, tag="ps2")
            a1, a2, a3 = dx * dx, dy * dy, dx * dy
            if a3 == 0:
                hc = 0.5 * (a1 - a2)
                nc.vector.tensor_scalar(pu2, c2, hc, 0.5, op0=OP.mult,
                                        op1=OP.add)
                nc.vector.tensor_scalar(ps2, c2, -hc, 0.5, op0=OP.mult,
                                        op1=OP.add)
            else:
                nc.vector.tensor_scalar(pu2, s2, float(a3), 1.0, op0=OP.mult,
                                        op1=OP.add)
                nc.vector.tensor_scalar(ps2, s2, float(-a3), 1.0, op0=OP.mult,
                                        op1=OP.add)
            nc.vector.tensor_tensor(pu2, pu2, iE, op=OP.mult)
            nc.vector.tensor_tensor(ps2, ps2, E, op=OP.mult)
            nc.vector.tensor_tensor(pu2, pu2, ps2, op=OP.add)
            nc.vector.tensor_tensor(pu2, pu2, rbp, op=OP.mult)
            kern = ptp.tile([128, R, H], F32, tag="kern")
            nc.scalar.activation(kern, pu2, AF.Exp, scale=-4.0)

            nc.vector.tensor_tensor(comu[ip][:, :, 1:1 + H], kern, Ctr(mu),
                                    op=OP.mult)
            cp = fld.tile([128, R, H], F32, tag=f"cp{ip}")
            nc.vector.tensor_tensor(
                cp, kern, MUSH[dx][:, :, 1 + dy:1 + dy + H], op=OP.mult)
            compat_t[ip] = cp
            if ip == 0:
                nc.vector.tensor_tensor(den, Ctr(mu), cp, op=OP.add)
            else:
                nc.vector.tensor_tensor(den, den, cp, op=OP.add)
            # mirror compat = comu shifted by (-dy, -dx)
            if dx != 0:
                cst = fld.tile([128, R, H + 2], F16, tag=f"csh{ip}")
                shift_into(cst, comu[ip], -dx, False, R)
                mirror = cst[:, :, 1 - dy:1 - dy + H]
            else:
                mirror = comu[ip][:, :, 1 - dy:1 - dy + H]
            mirror_t[ip] = mirror
            nc.vector.tensor_tensor(den, den, mirror, op=OP.add)

        if dbg is not None:
            nc.sync.dma_start(out=dbg["dbg_den"], in_=den)
        rden = fld.tile([128, R, H], F32, tag="rden")
        nc.scalar.activation(rden, den, AF.Ln, bias=cb[:, 2:3])
        nc.scalar.activation(rden, rden, AF.Exp, scale=-1.0)

        # --- normalized weights into Wt ---
        for ip, (dy, dx) in enumerate(PAIRS):
            nc.vector.tensor_tensor(Wt[:, DXI[dx], 1 + dy, :, 1:1 + H],
                                    compat_t[ip], rden, op=OP.mult)
            nc.vector.tensor_tensor(Wt[:, DXI[-dx], 1 - dy, :, 1:1 + H],
                                    mirror_t[ip], rden, op=OP.mult)
        nc.vector.tensor_tensor(Wt[:, 1, 1, :, 1:1 + H], Ctr(mu), rden,
                                op=OP.mult)
        if dbg is not None:
            nc.sync.dma_start(out=dbg["dbg_wt"], in_=Wt)

        # --- partition-shift dx groups: WQ[dx][g] = Wt[dx-group][g-dx] ---
        for dx, wq in ((1, WQp), (-1, WQm)):
            src = Wt[:, DXI[dx]].rearrange("p a r c -> p (a r) c")
            dst = wq.rearrange("p a r c -> p (a r) c")
            shift_into(dst, src, -dx, False, 3 * R)

    # ---------------- phase A2: z strips 1-3 (strip 0 was emitted before the
    # fields so its ACT evacs precede the field-ACT work and the stencil can
    # start the moment the fields finish) ----------
    for k in range(1, H // SA):
        emit_z_strip(k)

    # ---------------- stencil ----------------
    with tc.tile_pool(name="phC", bufs=2) as pc, \
         tc.tile_pool(name="psC", bufs=2, space="PSUM") as psC:
        for b in range(NBLK):
            r0 = b * BLK
            P = {}
            for dx in (-1, 0, 1):
                Pt = pc.tile([128, 3, R, Cout, BLK], F16, tag=f"P{dx}")
                for dyi, dy in enumerate((-1, 0, 1)):
                    wb = WQ[dx][:, dyi, :, None,
                                1 + r0:1 + r0 + BLK].to_broadcast(
                        [128, R, Cout, BLK])
                    kS = b // (SA // BLK)
                    c0 = r0 + dy - SA * kS + 1
                    nc.vector.tensor_tensor(
                        Pt[:, dyi],
                        zsv[kS][:, :, :, c0:c0 + BLK],
                        wb, op=OP.mult)
                P[dx] = Pt

            # acc memory is [Cout, BLK] so the matmul dst is CONTIGUOUS
            # (a transposed dst AP halves the PE stream rate); the ACT
            # evacuation un-transposes into [BLK, Cout] for the output DMA.
            acc = psC.tile([128, Cout, BLK], F32, tag="acc")
            n = 0
            for dx in (-1, 0, 1):
                for dyi in range(3):
                    for r in range(R):
                        rhs = P[dx][:, dyi, r]       # [128, 64, 8] contiguous
                        nc.tensor.matmul(acc, SHIFTH[dx], rhs,
                                         start=(n == 0), stop=False,
                                         skip_group_check=True)
                        n += 1
            nc.tensor.matmul(acc, aux[:, 0:128], aux[:, 128:640],
                             start=False, stop=True, skip_group_check=True)
            stg = pc.tile([128, BLK, Cout], F32, tag="stg")
            nc.scalar.activation(stg, acc.transpose([0, 2, 1]), AF.Copy)
            dst = out_d[r0 * 128:(r0 + BLK) * 128, :].rearrange(
                "(rho g) o -> g rho o", g=128)
            nc.sync.dma_start(out=dst, in_=stg)


def _host_prep(inputs):
    import ml_dtypes
    bf = ml_dtypes.bfloat16
    x = np.asarray(inputs["x"], np.float32)
    gate_w = np.asarray(inputs["gate_w"], np.float32)
    gate_b = np.asarray(inputs["gate_b"], np.float32)
    value_w = np.asarray(inputs["value_w"], np.float32)
    geom_w = np.asarray(inputs["geom_w"], np.float32)
    geom_b = np.asarray(inputs["geom_b"], np.float32)
    pw_w = np.asarray(inputs["pw_w"], np.float32)
    pw_b = np.asarray(inputs["pw_b"], np.float32)

    M = pw_w.reshape(Cout, R, C).transpose(1, 0, 2) @ value_w      # [R,Cout,C]
    wz = M.transpose(2, 0, 1).reshape(C, R * Cout)                 # [C, 256]
    wgq = np.concatenate([gate_w.T, geom_w.T], axis=1)             # [C, 16]

    xf = np.ascontiguousarray(x.reshape(B, C, L))
    xh = xf.astype(bf)
    xl = (xf - xh.astype(np.float32)).astype(bf)
    x2 = (xf - xh.astype(np.float32) - xl.astype(np.float32)).astype(bf)
    xslab = np.concatenate([xh, xl], axis=1)                       # [B,128,L]

    w2h = wgq.astype(bf)
    w2l = (wgq - w2h.astype(np.float32)).astype(bf)
    w2q = (wgq - w2h.astype(np.float32) - w2l.astype(np.float32)).astype(bf)
    wA = np.zeros((128, 304), np.float32)
    wA[0:64, 0:256] = wz
    wA[0:64, 256:272] = w2h.astype(np.float32)
    wA[64:128, 256:272] = w2l.astype(np.float32)
    wA[0:64, 272:288] = w2l.astype(np.float32)
    wA[64:128, 272:288] = w2h.astype(np.float32)
    wA[0:64, 288:304] = w2q.astype(np.float32)
    wA[64:128, 288:304] = w2q.astype(np.float32)

    smat = np.zeros((128, 384), np.float32)
    for g in range(128):   # Sp[k, g] = 1 iff k = g+1 ; Sm[k, g] = 1 iff k=g-1
        if g + 1 < 128:
            smat[g + 1, g] = 1.0
        if g - 1 >= 0:
            smat[g - 1, 128 + g] = 1.0
        smat[g, 256 + g] = 1.0

    aux = np.zeros((1, 640), np.float32)
    aux[0, 0:128] = 1.0
    aux[0, 128:640] = np.repeat(pw_b, BLK)   # acc is [Cout, BLK] o-major

    # z-pass: stationary holds [x_hi row 2p ; x_hi row 2p+1]
    xhr = xh.astype(np.float32).reshape(B, C, H, W)
    xz = np.concatenate([xhr[:, :, 0::2, :], xhr[:, :, 1::2, :]],
                        axis=1).reshape(B, 128, L // 2).astype(bf)
    wz2 = np.zeros((128, 512), np.float32)
    wz2[0:64, 0:256] = wz
    wz2[64:128, 256:512] = wz

    gqb_cols = np.concatenate([gate_b, geom_b])                    # [16]
    gqbias = np.tile(gqb_cols[None, :], (128, 1)).astype(np.float32)

    return {
        "xslab": xslab,
        "x2slab": x2,
        "xzslab": xz,
        "wz2": wz2.astype(bf),
        "wA": wA.astype(bf),
        "smat": smat[:, 0:256].copy(),
        "smath": smat.astype(np.float16),
        "gqbias": gqbias,
        "aux": aux.astype(bf),
    }


def make_in_maps(inputs):
    h = _host_prep(inputs)
    return [{"xslab": h["xslab"][b], "x2slab": h["x2slab"][b],
             "xzslab": h["xzslab"][b], "wz2": h["wz2"], "wA": h["wA"],
             "smat": h["smat"], "smath": h["smath"], "gqbias": h["gqbias"],
             "aux": h["aux"]} for b in range(B)]


def kernel(**inputs) -> np.ndarray:
    if "nc" not in _CACHE:
        _CACHE["nc"] = build_program()
    nc = _CACHE["nc"]
    in_maps = make_in_maps(inputs)
    res = run_bass_kernel_spmd(nc, in_maps, core_ids=list(range(NCORE)))
    out = np.stack([
        res.results[b]["out"].reshape(H, W, Cout).transpose(2, 0, 1)
        for b in range(B)
    ])
    return out.astype(np.float32)


# revision 35
# speedup vs baseline: 2.2180x; 1.0233x over previous
"""Trainium2 Bass kernel for nn_AZConv2d (fuzzy-rule hyperbolic-geometry message passing).

Self-contained: hardcodes shapes B=8,C=64,H=W=128,R=4,Cout=64; shards batch over 8 cores.

v7 (861us -> 400us vs v2): phase-split pipeline, all-bf16/fp16 matmul streams.
  - Pass A1 (gq only): per row one bf16 stationary [x_hi; x_lo] + 4 small
    matmuls giving gq = (x0+x1+x2)^T(w0+w1+w2) to ~fp32 accuracy (needed:
    theta pairs can be degenerate to ~1e-5); biases folded into the strip
    -granular PSUM->SBUF evac add. No fp32 LDWEIGHTS anywhere.
  - Fields: full-image [128, 4, 130] ops; Sin ops grouped before Exp/Ln ops
    (ACT table swaps); pair-loop temps double-buffered so pairs pipeline.
  - Pass A2 (z): per ROW-PAIR one N=512 matmul ([x0_j; x0_j+1] stationary vs
    block-diag [[wz,0],[0,wz]]); evacs ACT-only so the DVE stays free; z in
    4 strip tiles [128, 256ch, 34rho] fp16 (seam rows duplicated) so stencil
    products depend per-strip, not on the whole-image z; pass A2's pools stay
    open through the stencil (SBUF/PSUM space reuse would serialize phases).
    Its matmuls fill the PE while the DVE does fields; its evacs hide under
    the DVE-bound stencil.
  - Stencil: products on DVE (fp16 2x, ~16us/block is the wall) into
    P[dx][128, 3dy, 4r, 64o, 8rho], rho innermost everywhere; 36-term
    (dx,dy,r) sum as PSUM-accumulating shift matmuls with CONTIGUOUS rhs
    (N=512) and CONTIGUOUS dst acc[128, Cout, BLK] (a transposed matmul dst
    halves the PE rate: 429 vs 216ns measured); ACT evac un-transposes.
  - dedupe_ldweights(): the scheduler emits one LDWEIGHTS per matmul; repeats
    of the identical stationary are rewritten to NoOps post-schedule.
"""
import numpy as np
from contextlib import ExitStack

import concourse.bass as bass
import concourse.tile as tile
from concourse import mybir
from concourse.bass_utils import run_bass_kernel_spmd

F32 = mybir.dt.float32
F16 = mybir.dt.float16
BF16 = mybir.dt.bfloat16
AF = mybir.ActivationFunctionType
OP = mybir.AluOpType

B, C, H, W, R, Cout = 8, 64, 128, 128, 4, 64
L = H * W
NCORE = 8
BLK = 8                 # stencil rows per psum accumulation block
NBLK = H // BLK         # 16
SA = 32                 # phase A strip rows
PI = float(np.pi)
PAIRS = [(0, 1), (1, -1), (1, 0), (1, 1)]   # (dy, dx)
DXI = {-1: 0, 0: 1, 1: 2}

_CACHE = {}


def split_multiwaits(nc):
    """This walrus accepts ONE sync wait per instruction: split extras into
    same-engine NoOps inserted just before the instruction."""
    n = 0
    for bb in nc.main_func.blocks:
        out = []
        for ins in bb.instructions:
            si = ins.sync_info
            if si is not None and len(si.on_wait) > 1:
                waits = list(si.on_wait)
                for w in waits[:-1]:
                    n += 1
                    nop = mybir.InstNoOp(name=f"WSPLIT-{n}")
                    nop.engine = ins.engine
                    nop.sync_info = mybir.SyncInfo(on_wait=[w], on_update=[])
                    out.append(nop)
                ins.sync_info = mybir.SyncInfo(on_wait=[waits[-1]],
                                               on_update=list(si.on_update))
            out.append(ins)
        bb.instructions[:] = out
    return n


def dedupe_ldweights(nc):
    """The tile scheduler emits one LDWEIGHTS per matmul even when many
    consecutive matmuls stream against the identical stationary (e.g. the 12
    shift matmuls per dx group). A reload of the already-loaded array costs
    ~215ns and serializes with the stream. Convert LDWEIGHTS whose weight AP
    (and tile cfg) matches the previous PE weight load into NoOps, keeping
    sync_info so semaphore semantics are unchanged."""
    n = 0
    for bb in nc.main_func.blocks:
        last_sig = None
        out = []
        for ins in bb.instructions:
            tn = type(ins).__name__
            if tn == 'InstLdweights':
                sig = (str(ins.ins[0]),
                       str(getattr(ins, 'tile_position', None)),
                       str(getattr(ins, 'tile_size', None)),
                       str(getattr(ins, 'perf_mode', None)),
                       str(getattr(ins, 'is_transpose', None)))
                if sig == last_sig:
                    n += 1
                    nop = mybir.InstNoOp(name=f"LWDEDUP-{n}")
                    nop.engine = ins.engine
                    nop.sync_info = ins.sync_info
                    out.append(nop)
                    continue
                last_sig = sig
            out.append(ins)
        bb.instructions[:] = out
    return n


def build_program(debug=False):
    nc = bass.Bass()
    xslab_d = nc.dram_tensor("xslab", [128, L], BF16, kind="ExternalInput")
    x2_d = nc.dram_tensor("x2slab", [64, L], BF16, kind="ExternalInput")
    xz_d = nc.dram_tensor("xzslab", [128, L // 2], BF16, kind="ExternalInput")
    wz2_d = nc.dram_tensor("wz2", [128, 512], BF16, kind="ExternalInput")
    wA_d = nc.dram_tensor("wA", [128, 304], BF16, kind="ExternalInput")
    smat_d = nc.dram_tensor("smat", [128, 256], F32, kind="ExternalInput")
    smath_d = nc.dram_tensor("smath", [128, 384], F16, kind="ExternalInput")
    gqb_d = nc.dram_tensor("gqbias", [128, 16], F32, kind="ExternalInput")
    aux_d = nc.dram_tensor("aux", [1, 640], BF16, kind="ExternalInput")
    out_d = nc.dram_tensor("out", [L, Cout], F32, kind="ExternalOutput")
    dbg = None
    if debug:
        dbg = {
            "dbg_gq": nc.dram_tensor("dbg_gq", [128, 16, H + 2], F32,
                                     kind="ExternalOutput")[:],
            "dbg_z": nc.dram_tensor("dbg_z", [128, 256, H + 2], F16,
                                    kind="ExternalOutput")[:],
            "dbg_mu": nc.dram_tensor("dbg_mu", [128, R, H + 2], F32,
                                     kind="ExternalOutput")[:],
            "dbg_wt": nc.dram_tensor("dbg_wt", [128, 3, 3, R, H + 2], F16,
                                     kind="ExternalOutput")[:],
            "dbg_den": nc.dram_tensor("dbg_den", [128, R, H], F32,
                                      kind="ExternalOutput")[:],
        }

    with ExitStack() as ctx:
        tc = ctx.enter_context(tile.TileContext(nc))
        _emit(ctx, tc, xslab_d[:], x2_d[:], xz_d[:], wz2_d[:], wA_d[:],
              smat_d[:], smath_d[:], gqb_d[:], aux_d[:], out_d[:], dbg)
    ndup = dedupe_ldweights(nc)
    split_multiwaits(nc)
    if ndup == 0:
        log_msg = "dedupe_ldweights removed nothing"
    return nc


def _emit(ctx, tc, xslab_d, x2_d, xz_d, wz2_d, wA_d, smat_d, smath_d, gqb_d,
          aux_d, out_d, dbg=None):
    nc = tc.nc

    persist = ctx.enter_context(tc.tile_pool(name="persist", bufs=1))

    # ---------------- persistent tensors ----------------
    wA_sb = persist.tile([128, 304], BF16)
    nc.sync.dma_start(out=wA_sb, in_=wA_d)
    wz2_sb = persist.tile([128, 512], BF16)
    nc.sync.dma_start(out=wz2_sb, in_=wz2_d)
    smat = persist.tile([128, 256], F32)       # [Sp | Sm] f32
    nc.sync.dma_start(out=smat, in_=smat_d)
    smath = persist.tile([128, 384], F16)      # [Sp | Sm | I] fp16
    nc.sync.dma_start(out=smath, in_=smath_d)
    gqbias = persist.tile([128, 16], F32)
    nc.sync.dma_start(out=gqbias, in_=gqb_d)
    aux = persist.tile([1, 640], BF16)         # [ones(128) | pwb_row(512)]
    nc.sync.dma_start(out=aux, in_=aux_d)

    # bias constants for ACT ops ([P,1] APs)
    cb = persist.tile([128, 4], F32)
    nc.vector.memset(cb[:, 0:1], 1e-30)
    nc.vector.memset(cb[:, 1:2], 2e-4)
    nc.vector.memset(cb[:, 2:3], 1e-6)
    nc.vector.memset(cb[:, 3:4], float(np.pi / 2))

    SHIFT = {1: smat[:, 0:128], -1: smat[:, 128:256]}
    SHIFTH = {1: smath[:, 0:128], -1: smath[:, 128:256], 0: smath[:, 256:384]}

    # z in 4 strip tiles [128, 256 ch, 34 rho] fp16 (rho innermost; col c of
    # tile k = image row 32k-1+c, one halo row duplicated at each seam).
    # Strip granularity lets stencil products start as soon as their strip's
    # rows are evacuated instead of waiting for the whole-image z.
    zs = [persist.tile([128, 256, SA + 2], F16, name=f"zs{k}")
          for k in range(H // SA)]
    nc.vector.memset(zs[0][:, :, 0], 0.0)
    nc.vector.memset(zs[H // SA - 1][:, :, SA + 1], 0.0)
    zsv = [t.rearrange("p (r o) c -> p r o c", r=R) for t in zs]
    # gq: [128, 16 fields, 130] f32 (biases pre-added on evac; halo = 0)
    gq = persist.tile([128, 16, H + 2], F32)
    nc.vector.memset(gq[:, :, 0], 0.0)
    nc.vector.memset(gq[:, :, H + 1], 0.0)

    # normalized weights Wt[g, dxi, dyi, r, rho] fp16 + partition-shifted WQ
    Wt = persist.tile([128, 3, 3, R, H + 2], F16)
    WQp = persist.tile([128, 3, R, H + 2], F16)   # dx=+1 group shifted by -1
    WQm = persist.tile([128, 3, R, H + 2], F16)   # dx=-1 group shifted by +1
    WQ = {1: WQp, -1: WQm, 0: Wt[:, 1]}

    # ---------------- phase A2 machinery (z pass): pools open early and stay
    # open through the stencil (space reuse by the stencil P pool would
    # serialize phases). Strip 0 runs INSIDE the gq pass with DVE evacs (the
    # DVE is idle there); strips 1-3 run after the fields with ACT evacs that
    # hide under the DVE-bound stencil. ----------
    phz = ctx.enter_context(tc.tile_pool(name="phZ", bufs=2))
    psZ = ctx.enter_context(tc.tile_pool(name="psZ", bufs=4, space="PSUM"))

    def emit_z_strip(k, on_act):
        q0 = k * SA
        xzw = phz.tile([128, (SA // 2) * 128], BF16, tag="xzw", name="xzw")
        nc.sync.dma_start(
            out=xzw, in_=xz_d[:, (q0 // 2) * 128:(q0 // 2 + SA // 2) * 128])
        for p in range(SA // 2):
            pt2 = psZ.tile([128, 2, 256], F32, tag="pt2", name="pt2")
            nc.tensor.matmul(pt2, xzw[:, p * 128:(p + 1) * 128], wz2_sb,
                             start=True, stop=True, skip_group_check=True)
            srcT = pt2.transpose([0, 2, 1])               # [128, 256, 2]
            dst = zs[k][:, :, 1 + 2 * p:3 + 2 * p]
            if on_act:
                nc.scalar.activation(dst, srcT, AF.Copy)
            else:
                nc.vector.tensor_copy(dst, srcT)
            if p == 0 and k > 0:              # row 32k = prev tile's col 33
                nc.scalar.activation(zs[k - 1][:, :, SA + 1:SA + 2],
                                     srcT[:, :, 0:1], AF.Copy)
            if p == SA // 2 - 1 and k < H // SA - 1:
                # row 32k+31 = next tile's col 0
                nc.scalar.activation(zs[k + 1][:, :, 0:1],
                                     srcT[:, :, 1:2], AF.Copy)

    # ---------------- phase A1: gq (z strip 0 interleaved) ----------------
    with tc.tile_pool(name="phG", bufs=2) as phg, \
         tc.tile_pool(name="psG", bufs=2, space="PSUM") as psG:
        for k in range(H // SA):
            q0 = k * SA
            xw = phg.tile([128, SA * 128], BF16, tag="xw")
            nc.sync.dma_start(out=xw, in_=xslab_d[:, q0 * 128:(q0 + SA) * 128])
            xw2 = phg.tile([64, SA * 128], BF16, tag="xw2")
            nc.sync.dma_start(out=xw2, in_=x2_d[:, q0 * 128:(q0 + SA) * 128])
            gqp = psG.tile([128, SA * 16], F32, tag="gqp")
            for j in range(SA):
                lhsT = xw[:, j * 128:(j + 1) * 128]
                g16 = gqp[:, j * 16:(j + 1) * 16]
                # gq = (x0+x1+x2)^T (w0+w1+w2) to ~fp32 accuracy:
                # [w0;w1]+[w1;w0]+[w2;w2] on [x0;x1], then x2^T w0.
                nc.tensor.matmul(g16, lhsT, wA_sb[:, 256:272],
                                 start=True, stop=False,
                                 skip_group_check=True)
                nc.tensor.matmul(g16, lhsT, wA_sb[:, 272:288],
                                 start=False, stop=False,
                                 skip_group_check=True)
                nc.tensor.matmul(g16, lhsT, wA_sb[:, 288:304],
                                 start=False, stop=False,
                                 skip_group_check=True)
                nc.tensor.matmul(g16, xw2[:, j * 128:(j + 1) * 128],
                                 wA_sb[0:64, 256:272],
                                 start=False, stop=True,
                                 skip_group_check=True)
            # gq strip evac with bias add: psum [32, 16] -> gq [16, 32]
            dstg = gq[:, :, 1 + q0:1 + q0 + SA]
            srcg = gqp.rearrange("p (j c) -> p c j", c=16)
            bcol = gqbias[:, :, None].to_broadcast([128, 16, SA])
            nc.vector.tensor_tensor(dstg, srcg, bcol, op=OP.add)
            if k == 0:
                emit_z_strip(0, on_act=False)

    if dbg is not None:
        nc.sync.dma_start(out=dbg["dbg_gq"], in_=gq)

    # ---------------- fields (full image) ----------------
    with tc.tile_pool(name="phF", bufs=1) as fld, \
         tc.tile_pool(name="psF", bufs=4, space="PSUM") as psF:
        fst = ctx.enter_context(tc.tile_pool(name="fsetup", bufs=1))

        def shift_into(dst_t, src_ap, sgn, dtype_f32, nch):
            """dst[g] = src[g+sgn]; src/dst [128, nch, 130]; 2-rule chunks."""
            step = 2
            for c0 in range(0, nch, step):
                ps = psF.tile([128, step, H + 2], F32, tag="psh")
                if dtype_f32:
                    nc.tensor.matmul(ps, SHIFT[sgn], src_ap[:, c0:c0 + step],
                                     start=True, stop=True,
                                     skip_group_check=True)
                else:
                    nc.tensor.matmul(ps, SHIFTH[sgn], src_ap[:, c0:c0 + step],
                                     start=True, stop=True,
                                     skip_group_check=True)
                nc.scalar.activation(dst_t[:, c0:c0 + step], ps, AF.Copy)

        thw = gq[:, 4:8, :]     # theta + b_th
        # --- theta path first (Sin table) ---
        m1 = fst.tile([128, R, H + 2], F32, tag="m1")
        m2 = fst.tile([128, R, H + 2], F32, tag="m2")
        tred = fst.tile([128, R, H + 2], F32, tag="tred")
        tred2 = fst.tile([128, R, H + 2], F32, tag="tred2")
        s2cF = fld.tile([128, R, H + 2], F32, tag="s2cF")
        c2cF = fld.tile([128, R, H + 2], F32, tag="c2cF")
        nc.vector.tensor_scalar(m1, thw, -PI / 2, None, op0=OP.is_lt)
        nc.vector.tensor_scalar(m2, thw, PI / 2, None, op0=OP.is_gt)
        nc.vector.tensor_tensor(m1, m1, m2, op=OP.subtract)
        nc.vector.scalar_tensor_tensor(out=tred, in0=m1, scalar=PI, in1=thw,
                                       op0=OP.mult, op1=OP.add)
        nc.scalar.activation(s2cF, tred, AF.Sin, scale=2.0)
        nc.vector.tensor_scalar(m1, thw, -0.75 * PI, None, op0=OP.is_lt)
        nc.vector.tensor_scalar(m2, thw, 0.25 * PI, None, op0=OP.is_gt)
        nc.vector.tensor_tensor(m1, m1, m2, op=OP.subtract)
        nc.vector.scalar_tensor_tensor(out=tred2, in0=m1, scalar=PI, in1=thw,
                                       op0=OP.mult, op1=OP.add)
        nc.scalar.activation(c2cF, tred2, AF.Sin, scale=2.0, bias=cb[:, 3:4])

        # --- softmax mu (Exp/Ln table) ---
        eg = fst.tile([128, R, H + 2], F32, tag="eg")
        nc.scalar.activation(eg, gq[:, 0:4, :], AF.Exp)
        nc.vector.memset(eg[:, :, 0], 0.0)
        nc.vector.memset(eg[:, :, H + 1], 0.0)
        zsum = fst.tile([128, H + 2], F32, tag="zsum")
        nc.vector.tensor_tensor(zsum, eg[:, 0], eg[:, 1], op=OP.add)
        nc.vector.tensor_tensor(zsum, zsum, eg[:, 2], op=OP.add)
        nc.vector.tensor_tensor(zsum, zsum, eg[:, 3], op=OP.add)
        rz = fst.tile([128, H + 2], F32, tag="rz")
        nc.scalar.activation(rz, zsum, AF.Ln, bias=cb[:, 0:1])
        nc.scalar.activation(rz, rz, AF.Exp, scale=-1.0)
        mu = fld.tile([128, R, H + 2], F32, tag="mu")
        rzb = rz[:, None, :].to_broadcast([128, R, H + 2])
        nc.vector.tensor_tensor(mu, eg, rzb, op=OP.mult)

        # --- hyper / base fields ---
        uh = fst.tile([128, R, H + 2], F32, tag="uh")
        nc.scalar.activation(uh, gq[:, 12:16, :], AF.Exp)
        ub = fst.tile([128, R, H + 2], F32, tag="ub")
        nc.scalar.activation(ub, gq[:, 8:12, :], AF.Exp)
        Ft = fld.tile([128, R, H + 2], F32, tag="Ft")
        nc.vector.tensor_scalar_add(Ft, uh, 1.0)
        lnf = fst.tile([128, R, H + 2], F32, tag="lnf")
        nc.scalar.activation(lnf, uh, AF.Ln, bias=1.0)
        Gt = fld.tile([128, R, H + 2], F32, tag="Gt")
        nc.scalar.activation(Gt, lnf, AF.Exp, scale=-1.0)
        bt = fld.tile([128, R, H + 2], F32, tag="bt")
        nc.scalar.activation(bt, ub, AF.Ln, bias=1.0)

        if dbg is not None:
            nc.sync.dma_start(out=dbg["dbg_mu"], in_=mu)

        # --- shifted copies ---
        base = {"c2c": c2cF, "s2c": s2cF, "Ft": Ft, "Gt": Gt, "bt": bt}
        shifted = {}
        for name, t in base.items():
            d = {0: t}
            for sgn in (1, -1):
                st = fld.tile([128, R, H + 2], F32, tag=f"{name}s{sgn}")
                shift_into(st, t, sgn, True, R)
                d[sgn] = st
            shifted[name] = d
        mu16 = fld.tile([128, R, H + 2], F16, tag="mu16")
        nc.vector.tensor_copy(mu16, mu)
        mup = fld.tile([128, R, H + 2], F16, tag="mup")
        mum = fld.tile([128, R, H + 2], F16, tag="mum")
        shift_into(mup, mu16, 1, False, R)
        shift_into(mum, mu16, -1, False, R)
        MUSH = {0: mu16, 1: mup, -1: mum}

        ptp = ctx.enter_context(tc.tile_pool(name="ptmp", bufs=2))

        # --- pair loop ---
        comu = [fld.tile([128, R, H + 2], F16, tag=f"comu{i}",
                         name=f"comu{i}") for i in range(4)]
        for cm in comu:
            nc.vector.memset(cm[:, :, 0], 0.0)
            nc.vector.memset(cm[:, :, H + 1], 0.0)
        den = fld.tile([128, R, H], F32, tag="den")
        compat_t = {}
        mirror_t = {}

        def Ctr(t):
            return t[:, :, 1:1 + H]

        for ip, (dy, dx) in enumerate(PAIRS):
            def S(name):
                return shifted[name][dx][:, :, 1 + dy:1 + dy + H]

            c2 = ptp.tile([128, R, H], F32, tag="c2")
            s2 = ptp.tile([128, R, H], F32, tag="s2")
            q = ptp.tile([128, R, H], F32, tag="q")
            t1 = ptp.tile([128, R, H], F32, tag="t1")
            nc.vector.tensor_tensor(c2, Ctr(c2cF), S("c2c"), op=OP.add)
            nc.vector.tensor_tensor(s2, Ctr(s2cF), S("s2c"), op=OP.add)
            nc.vector.tensor_tensor(q, c2, c2, op=OP.mult)
            nc.vector.tensor_tensor(t1, s2, s2, op=OP.mult)
            nc.vector.tensor_tensor(q, q, t1, op=OP.add)
            rin = ptp.tile([128, R, H], F32, tag="rin")
            nc.scalar.activation(rin, q, AF.Ln)
            nc.scalar.activation(rin, rin, AF.Exp, scale=-0.5)
            nc.vector.tensor_scalar(rin, rin, 1e6, None, op0=OP.min)
            nc.vector.tensor_tensor(c2, c2, rin, op=OP.mult)
            nc.vector.tensor_tensor(s2, s2, rin, op=OP.mult)
            E = ptp.tile([128, R, H], F32, tag="E")
            iE = ptp.tile([128, R, H], F32, tag="iE")
            bp = ptp.tile([128, R, H], F32, tag="bp")
            nc.vector.tensor_tensor(E, Ctr(Ft), S("Ft"), op=OP.mult)
            nc.vector.tensor_tensor(iE, Ctr(Gt), S("Gt"), op=OP.mult)
            nc.vector.tensor_tensor(bp, Ctr(bt), S("bt"), op=OP.add)
            rbp = ptp.tile([128, R, H], F32, tag="rbp")
            nc.scalar.activation(rbp, bp, AF.Ln, bias=cb[:, 1:2])
            nc.scalar.activation(rbp, rbp, AF.Exp, scale=-2.0)
            pu2 = ptp.tile([128, R, H], F32, tag="pu2")
            ps2 = ptp.tile([128, R, H], F32, tag="ps2")
            a1, a2, a3 = dx * dx, dy * dy, dx * dy
            if a3 == 0:
                hc = 0.5 * (a1 - a2)
                nc.vector.tensor_scalar(pu2, c2, hc, 0.5, op0=OP.mult,
                                        op1=OP.add)
                nc.vector.tensor_scalar(ps2, c2, -hc, 0.5, op0=OP.mult,
                                        op1=OP.add)
            else:
                nc.vector.tensor_scalar(pu2, s2, float(a3), 1.0, op0=OP.mult,
                                        op1=OP.add)
                nc.vector.tensor_scalar(ps2, s2, float(-a3), 1.0, op0=OP.mult,
                                        op1=OP.add)
            nc.vector.tensor_tensor(pu2, pu2, iE, op=OP.mult)
            nc.vector.tensor_tensor(ps2, ps2, E, op=OP.mult)
            nc.vector.tensor_tensor(pu2, pu2, ps2, op=OP.add)
            nc.vector.tensor_tensor(pu2, pu2, rbp, op=OP.mult)
            kern = ptp.tile([128, R, H], F32, tag="kern")
            nc.scalar.activation(kern, pu2, AF.Exp, scale=-4.0)

            nc.vector.tensor_tensor(comu[ip][:, :, 1:1 + H], kern, Ctr(mu),
                                    op=OP.mult)
            cp = fld.tile([128, R, H], F32, tag=f"cp{ip}")
            nc.vector.tensor_tensor(
                cp, kern, MUSH[dx][:, :, 1 + dy:1 + dy + H], op=OP.mult)
            compat_t[ip] = cp
            if ip == 0:
                nc.vector.tensor_tensor(den, Ctr(mu), cp, op=OP.add)
            else:
                nc.vector.tensor_tensor(den, den, cp, op=OP.add)
            # mirror compat = comu shifted by (-dy, -dx)
            if dx != 0:
                cst = fld.tile([128, R, H + 2], F16, tag=f"csh{ip}")
                shift_into(cst, comu[ip], -dx, False, R)
                mirror = cst[:, :, 1 - dy:1 - dy + H]
            else:
                mirror = comu[ip][:, :, 1 - dy:1 - dy + H]
            mirror_t[ip] = mirror
            nc.vector.tensor_tensor(den, den, mirror, op=OP.add)

        if dbg is not None:
            nc.sync.dma_start(out=dbg["dbg_den"], in_=den)
        rden = fld.tile([128, R, H], F32, tag="rden")
        nc.scalar.activation(rden, den, AF.Ln, bias=cb[:, 2:3])
        nc.scalar.activation(rden, rden, AF.Exp, scale=-1.0)

        # --- normalized weights into Wt ---
        for ip, (dy, dx) in enumerate(PAIRS):
            nc.vector.tensor_tensor(Wt[:, DXI[dx], 1 + dy, :, 1:1 + H],
                                    compat_t[ip], rden, op=OP.mult)
            nc.vector.tensor_tensor(Wt[:, DXI[-dx], 1 - dy, :, 1:1 + H],
                                    mirror_t[ip], rden, op=OP.mult)
        nc.vector.tensor_tensor(Wt[:, 1, 1, :, 1:1 + H], Ctr(mu), rden,
                                op=OP.mult)
        if dbg is not None:
            nc.sync.dma_start(out=dbg["dbg_wt"], in_=Wt)

        # --- partition-shift dx groups: WQ[dx][g] = Wt[dx-group][g-dx] ---
        for dx, wq in ((1, WQp), (-1, WQm)):
            src = Wt[:, DXI[dx]].rearrange("p a r c -> p (a r) c")
            dst = wq.rearrange("p a r c -> p (a r) c")
            shift_into(dst, src, -dx, False, 3 * R)

    # ---------------- phase A2: z strips 1-3 (strip 0 was emitted before the
    # fields so its ACT evacs precede the field-ACT work and the stencil can
    # start the moment the fields finish) ----------
    for k in range(1, H // SA):
        emit_z_strip(k, on_act=True)

    # ---------------- stencil ----------------
    with tc.tile_pool(name="phC", bufs=2) as pc, \
         tc.tile_pool(name="psC", bufs=2, space="PSUM") as psC:
        for b in range(NBLK):
            r0 = b * BLK
            P = {}
            for dx in (-1, 0, 1):
                Pt = pc.tile([128, 3, R, Cout, BLK], F16, tag=f"P{dx}")
                for dyi, dy in enumerate((-1, 0, 1)):
                    wb = WQ[dx][:, dyi, :, None,
                                1 + r0:1 + r0 + BLK].to_broadcast(
                        [128, R, Cout, BLK])
                    kS = b // (SA // BLK)
                    c0 = r0 + dy - SA * kS + 1
                    nc.vector.tensor_tensor(
                        Pt[:, dyi],
                        zsv[kS][:, :, :, c0:c0 + BLK],
                        wb, op=OP.mult)
                P[dx] = Pt

            # acc memory is [Cout, BLK] so the matmul dst is CONTIGUOUS
            # (a transposed dst AP halves the PE stream rate); the ACT
            # evacuation un-transposes into [BLK, Cout] for the output DMA.
            acc = psC.tile([128, Cout, BLK], F32, tag="acc")
            n = 0
            for dx in (-1, 0, 1):
                for dyi in range(3):
                    for r in range(R):
                        rhs = P[dx][:, dyi, r]       # [128, 64, 8] contiguous
                        nc.tensor.matmul(acc, SHIFTH[dx], rhs,
                                         start=(n == 0), stop=False,
                                         skip_group_check=True)
                        n += 1
            nc.tensor.matmul(acc, aux[:, 0:128], aux[:, 128:640],
                             start=False, stop=True, skip_group_check=True)
            stg = pc.tile([128, BLK, Cout], F32, tag="stg")
            nc.scalar.activation(stg, acc.transpose([0, 2, 1]), AF.Copy)
            dst = out_d[r0 * 128:(r0 + BLK) * 128, :].rearrange(
                "(rho g) o -> g rho o", g=128)
            nc.sync.dma_start(out=dst, in_=stg)


def _host_prep(inputs):
    import ml_dtypes
    bf = ml_dtypes.bfloat16
    x = np.asarray(inputs["x"], np.float32)
    gate_w = np.asarray(inputs["gate_w"], np.float32)
    gate_b = np.asarray(inputs["gate_b"], np.float32)
    value_w = np.asarray(inputs["value_w"], np.float32)
    geom_w = np.asarray(inputs["geom_w"], np.float32)
    geom_b = np.asarray(inputs["geom_b"], np.float32)
    pw_w = np.asarray(inputs["pw_w"], np.float32)
    pw_b = np.asarray(inputs["pw_b"], np.float32)

    M = pw_w.reshape(Cout, R, C).transpose(1, 0, 2) @ value_w      # [R,Cout,C]
    wz = M.transpose(2, 0, 1).reshape(C, R * Cout)                 # [C, 256]
    wgq = np.concatenate([gate_w.T, geom_w.T], axis=1)             # [C, 16]

    xf = np.ascontiguousarray(x.reshape(B, C, L))
    xh = xf.astype(bf)
    xl = (xf - xh.astype(np.float32)).astype(bf)
    x2 = (xf - xh.astype(np.float32) - xl.astype(np.float32)).astype(bf)
    xslab = np.concatenate([xh, xl], axis=1)                       # [B,128,L]

    w2h = wgq.astype(bf)
    w2l = (wgq - w2h.astype(np.float32)).astype(bf)
    w2q = (wgq - w2h.astype(np.float32) - w2l.astype(np.float32)).astype(bf)
    wA = np.zeros((128, 304), np.float32)
    wA[0:64, 0:256] = wz
    wA[0:64, 256:272] = w2h.astype(np.float32)
    wA[64:128, 256:272] = w2l.astype(np.float32)
    wA[0:64, 272:288] = w2l.astype(np.float32)
    wA[64:128, 272:288] = w2h.astype(np.float32)
    wA[0:64, 288:304] = w2q.astype(np.float32)
    wA[64:128, 288:304] = w2q.astype(np.float32)

    smat = np.zeros((128, 384), np.float32)
    for g in range(128):   # Sp[k, g] = 1 iff k = g+1 ; Sm[k, g] = 1 iff k=g-1
        if g + 1 < 128:
            smat[g + 1, g] = 1.0
        if g - 1 >= 0:
            smat[g - 1, 128 + g] = 1.0
        smat[g, 256 + g] = 1.0

    aux = np.zeros((1, 640), np.float32)
    aux[0, 0:128] = 1.0
    aux[0, 128:640] = np.repeat(pw_b, BLK)   # acc is [Cout, BLK] o-major

    # z-pass: stationary holds [x_hi row 2p ; x_hi row 2p+1]
    xhr = xh.astype(np.float32).reshape(B, C, H, W)
    xz = np.concatenate([xhr[:, :, 0::2, :], xhr[:, :, 1::2, :]],
                        axis=1).reshape(B, 128, L // 2).astype(bf)
    wz2 = np.zeros((128, 512), np.float32)
    wz2[0:64, 0:256] = wz
    wz2[64:128, 256:512] = wz

    gqb_cols = np.concatenate([gate_b, geom_b])                    # [16]
    gqbias = np.tile(gqb_cols[None, :], (128, 1)).astype(np.float32)

    return {
        "xslab": xslab,
        "x2slab": x2,
        "xzslab": xz,
        "wz2": wz2.astype(bf),
        "wA": wA.astype(bf),
        "smat": smat[:, 0:256].copy(),
        "smath": smat.astype(np.float16),
        "gqbias": gqbias,
        "aux": aux.astype(bf),
    }


def make_in_maps(inputs):
    h = _host_prep(inputs)
    return [{"xslab": h["xslab"][b], "x2slab": h["x2slab"][b],
             "xzslab": h["xzslab"][b], "wz2": h["wz2"], "wA": h["wA"],
             "smat": h["smat"], "smath": h["smath"], "gqbias": h["gqbias"],
             "aux": h["aux"]} for b in range(B)]


def kernel(**inputs) -> np.ndarray:
    if "nc" not in _CACHE:
        _CACHE["nc"] = build_program()
    nc = _CACHE["nc"]
    in_maps = make_in_maps(inputs)
    res = run_bass_kernel_spmd(nc, in_maps, core_ids=list(range(NCORE)))
    out = np.stack([
        res.results[b]["out"].reshape(H, W, Cout).transpose(2, 0, 1)
        for b in range(B)
    ])
    return out.astype(np.float32)
